# revision 29
# baseline (speedup 1.0000x reference)
"""Causal self-attention (B=4, T=2048, C=1024, H=16, D=64) on 8 TRN2 NeuronCores.

Sharding: core c handles batch b = c//2 and head-half hh = c%2 (8 of 16 heads).
Each core computes its partial c_proj output [T, C] in bf16; the host sums the
two partials per batch and adds b_proj + b_v @ w_proj (the v-bias commutes
through the attention average since softmax weights sum to 1).

v3: single dense PE instruction stream instead of serial phases.
  - qb-major attention with a chunk-granular software pipeline:
    S^T chunk (2 matmuls, head pair) -> exp (ScalarE) -> PV lagging LAG chunks.
  - Head-pair row-packing: even head uses PE rows 0-63, odd head rows 64-127
    (auto tile_position) so the two K=64 S^T matmuls run concurrently on HW.
  - QKV projection / c_proj groups interleaved between attention chunks as
    filler so PE stays busy while ScalarE exponentiates.
  - Softmax denominator rides the PV matmul as a ones-column in the V tile;
    normalize is a lean bf16 DVE chain (copy/recip/bcast/mult), GpSimd does
    the partition broadcast.
  - Diagonal chunks exp both head-halves in ONE ScalarE instruction (3D AP).
  - wqkQ/wqkK packed head-pair-major so the prologue consumes the DMA stream
    sequentially; first matmul starts as soon as ~160KB has landed. PE
    warm-up matmuls run during the initial DMA latency window (HAM ramp).
  - bf16 output (halved eviction + DMA-out cost); host accumulates in f32.
"""

import sys
from collections import deque

for _p in ("/opt/trn_rl_repo", "/root/.axon_site"):
    if _p not in sys.path:
        sys.path.append(_p)

import numpy as np
import ml_dtypes

BF16 = ml_dtypes.bfloat16

B, T, C, H = 4, 2048, 1024, 16
D = C // H          # 64
NCORES = 8
HL = H // 2         # 8 local heads
NP = HL // 2        # 4 head pairs
CL = HL * D         # 512 local qkv channels
P = 128
TT = T // P         # 16 token tiles
QB = 512            # query block
NQB = T // QB       # 4
NCT = C // P        # 8 contraction tiles

LAG = 4
FINE_INTERLEAVE = True   # pull filler MMs between chunks (vs whole groups at block ends)
USE_LATE = True          # scalar-copy evicts + ps_s borrowing for tail proj groups
TAIL_RESERVE_MMS = 48
NCHUNK_BUFS = 8

_BUILT = None


def _build():
    import concourse.mybir as mybir
    import concourse.tile as tile
    from concourse import bacc
    from concourse.masks import make_upper_triangular

    bf = mybir.dt.bfloat16
    f32 = mybir.dt.float32

    nc = bacc.Bacc("TRN2", target_bir_lowering=False, debug=False,
                   num_devices=NCORES)

    # xT[tch][p, kc*512 + t]: token chunk tch, contraction block kc
    xT_d = nc.dram_tensor("xT", [NQB, P, NCT * QB], bf, kind="ExternalInput")
    # wqk*[p, hb*1024 + kc*128 + d]: head-pair major so the prologue's
    # hb-ordered K groups consume the DMA stream sequentially
    wqkQ_d = nc.dram_tensor("wqkQ", [P, NCT * CL], bf, kind="ExternalInput")
    wqkK_d = nc.dram_tensor("wqkK", [P, NCT * CL], bf, kind="ExternalInput")
    # wv[p, kc*512 + oc]: contraction block kc, local out channel oc
    wv_d = nc.dram_tensor("wv", [P, NCT * CL], bf, kind="ExternalInput")
    bqk_d = nc.dram_tensor("bqk", [P, (2 * CL) // P], f32, kind="ExternalInput")
    # wp[p, j*1024 + oc]
    wp_d = nc.dram_tensor("wp", [P, (CL // P) * C], bf, kind="ExternalInput")
    out_d = nc.dram_tensor("out", [TT, P, C], bf, kind="ExternalOutput")
    import os
    DBG = bool(os.environ.get("K2_DEBUG"))
    if DBG:
        dbg_qT = nc.dram_tensor("dbg_qT", [NP, P, T], f32, kind="ExternalOutput")
        dbg_kT = nc.dram_tensor("dbg_kT", [NP, P, T], f32, kind="ExternalOutput")
        dbg_v = nc.dram_tensor("dbg_v", [TT, P, HL * (D + 1)], f32,
                               kind="ExternalOutput")
        dbg_yT = nc.dram_tensor("dbg_yT", [CL // P, P, T], f32,
                                kind="ExternalOutput")

    with tile.TileContext(nc) as tc:
        with (
            tc.tile_pool(name="const", bufs=1) as const,
            tc.tile_pool(name="weights", bufs=1) as wpool,
            tc.tile_pool(name="acts", bufs=1) as apool,
            tc.tile_pool(name="chunks", bufs=NCHUNK_BUFS) as cpool,
            tc.tile_pool(name="outsb", bufs=6) as opool,
            tc.tile_pool(name="small", bufs=4) as small,
            tc.tile_pool(name="dbgpool", bufs=2) as dpool,
            tc.tile_pool(name="ps_s", bufs=2, space="PSUM") as ps_s,
            tc.tile_pool(name="ps_y", bufs=2, space="PSUM") as ps_y,
            tc.tile_pool(name="ps_mm", bufs=2, space="PSUM") as ps_mm,
        ):
            # ---- SBUF tensors ----
            wqkQ_sb = wpool.tile([P, NCT * CL], bf, tag="wqkQ", name="wqkQ")
            wqkK_sb = wpool.tile([P, NCT * CL], bf, tag="wqkK", name="wqkK")
            wv_sb = wpool.tile([P, NCT * CL], bf, tag="wv", name="wv")
            wp_sb = wpool.tile([P, (CL // P) * C], bf, tag="wp", name="wp")
            xT_sb = [apool.tile([P, NCT * QB], bf, tag=f"xTt{t}", name=f"xTt{t}")
                     for t in range(NQB)]
            qT_sb = [apool.tile([P, T], bf, tag=f"qT{b}", name=f"qT{b}") for b in range(NP)]
            kT_sb = [apool.tile([P, T], bf, tag=f"kT{b}", name=f"kT{b}") for b in range(NP)]
            v65_sb = [apool.tile([P, HL * (D + 1)], bf, tag=f"v65{ti}", name=f"v65{ti}")
                      for ti in range(TT)]
            yT_sb = [apool.tile([P, T], bf, tag=f"yT{j}", name=f"yT{j}") for j in range(CL // P)]

            # ---- DMA emission (ordered for earliest compute start) ----
            # wqkK is hb-major: cols [hb*1024, (hb+1)*1024) cover one whole
            # k_group, consumed kc-sequentially. xT[0] feeds ALL four
            # K groups, so it streams at priority right after the first
            # small slices; later-hb weight blocks arrive during group 0.
            bqk_sb = const.tile([P, (2 * CL) // P], f32, tag="bqk")
            nc.sync.dma_start(out=wqkK_sb[:, 0:128], in_=wqkK_d.ap()[:, 0:128])
            nc.sync.dma_start(out=xT_sb[0][:, 0:512], in_=xT_d.ap()[0][:, 0:512])
            nc.sync.dma_start(out=bqk_sb[:], in_=bqk_d.ap())
            nc.sync.dma_start(out=wqkK_sb[:, 128:1024],
                              in_=wqkK_d.ap()[:, 128:1024])
            nc.sync.dma_start(out=xT_sb[0][:, 512:1024],
                              in_=xT_d.ap()[0][:, 512:1024])
            nc.sync.dma_start(out=xT_sb[0][:, 1024:2048],
                              in_=xT_d.ap()[0][:, 1024:2048])
            nc.sync.dma_start(out=xT_sb[0][:, 2048:4096],
                              in_=xT_d.ap()[0][:, 2048:4096])
            nc.sync.dma_start(out=wqkK_sb[:, 1024:2048],
                              in_=wqkK_d.ap()[:, 1024:2048])
            nc.sync.dma_start(out=wqkK_sb[:, 2048:4096],
                              in_=wqkK_d.ap()[:, 2048:4096])

            # ---- constants / act-table preload (off the critical DMA path) ----
            warm = const.tile([1, 16], f32, tag="warm")
            nc.vector.memset(warm[:], 0.0)
            wexp = const.tile([1, 16], f32, tag="wexp")
            nc.scalar.activation(out=wexp[:], in_=warm[:],
                                 func=mybir.ActivationFunctionType.Exp)
            # PE warm-up during the initial DMA latency window: keeps the
            # HAM activity monitor busy so real matmuls start at full clock.
            warm_bf = const.tile([1, 64], bf, tag="warmbf")
            nc.vector.memset(warm_bf[:], 0.0)
            ps_warm = ps_mm.tile([P, QB], f32, tag="mm", name="warmps")
            for _ in range(12):
                nc.tensor.matmul(ps_warm[0:64, 0:64],
                                 lhsT=warm_bf[:, 0:64],
                                 rhs=warm_bf[:, 0:64],
                                 start=True, stop=True)
            umask = const.tile([P, P], bf, tag="umask")
            make_upper_triangular(nc, umask[:], val=1.0, diag=True)
            nc.sync.dma_start(out=wv_sb[:], in_=wv_d.ap())
            nc.sync.dma_start(out=wqkQ_sb[:], in_=wqkQ_d.ap())
            for tch in range(1, NQB):
                nc.sync.dma_start(out=xT_sb[tch][:], in_=xT_d.ap()[tch])
            nc.sync.dma_start(out=wp_sb[:], in_=wp_d.ap())

            # ---- group emitters (each: list of MM closures + evict) ----
            def k_group(hb, tch):
                ps = [None]
                def mk(kc):
                    def mm():
                        if ps[0] is None:
                            ps[0] = ps_mm.tile([P, QB], f32, tag="mm", name="mm")
                        nc.tensor.matmul(
                            ps[0][:],
                            lhsT=wqkK_sb[:, hb * (NCT * P) + kc * P:
                                            hb * (NCT * P) + (kc + 1) * P],
                            rhs=xT_sb[tch][:, kc * QB:(kc + 1) * QB],
                            start=(kc == 0), stop=(kc == NCT - 1))
                    return mm
                def evict():
                    nc.vector.tensor_scalar_add(
                        kT_sb[hb][:, tch * QB:(tch + 1) * QB],
                        ps[0][:], bqk_sb[:, NP + hb:NP + hb + 1])
                return [mk(kc) for kc in range(NCT)], evict

            def q_group(hb, tch):
                ps = [None]
                def mk(kc):
                    def mm():
                        if ps[0] is None:
                            ps[0] = ps_mm.tile([P, QB], f32, tag="mm", name="mm")
                        nc.tensor.matmul(
                            ps[0][:],
                            lhsT=wqkQ_sb[:, hb * (NCT * P) + kc * P:
                                            hb * (NCT * P) + (kc + 1) * P],
                            rhs=xT_sb[tch][:, kc * QB:(kc + 1) * QB],
                            start=(kc == 0), stop=(kc == NCT - 1))
                    return mm
                def evict():
                    nc.vector.tensor_scalar_add(
                        qT_sb[hb][:, tch * QB:(tch + 1) * QB],
                        ps[0][:], bqk_sb[:, hb:hb + 1])
                return [mk(kc) for kc in range(NCT)], evict

            def v_group(ti):
                ps = [None]
                def mk(kc):
                    def mm():
                        if ps[0] is None:
                            ps[0] = ps_mm.tile([P, QB], f32, tag="mm", name="mm")
                        nc.tensor.matmul(
                            ps[0][:],
                            lhsT=xT_sb[ti // 4][:, kc * QB + (ti % 4) * P:
                                                  kc * QB + (ti % 4 + 1) * P],
                            rhs=wv_sb[:, kc * CL:(kc + 1) * CL],
                            start=(kc == 0), stop=(kc == NCT - 1))
                    return mm
                def evict():
                    # v65 layout per head: cols 0..63 = v, col 64 = ones (the
                    # softmax denominator rides the PV matmul). v-bias is
                    # folded into the host-side output bias (sum_k p_k = 1).
                    v3 = v65_sb[ti][:].rearrange("p (h e) -> p h e", e=D + 1)
                    nc.vector.tensor_copy(
                        out=v3[:, :, 0:D],
                        in_=ps[0][:].rearrange("p (h e) -> p h e", e=D))
                    nc.vector.memset(v3[:, :, D:D + 1], 1.0)
                return [mk(kc) for kc in range(NCT)], evict

            def proj_group(ti, co, late=False, split=False):
                ps = [None]
                def mk(j):
                    def mm():
                        if ps[0] is None:
                            if USE_LATE and late == 2 and (ti * 2 + co) % 2 == 0:
                                big = ps_s.tile([P, 2 * QB], f32, tag="s",
                                                name="pss")
                                ps[0] = big[:, 0:QB]
                            else:
                                ps[0] = ps_mm.tile([P, QB], f32, tag="mm",
                                                   name="mm")
                        nc.tensor.matmul(
                            ps[0][:],
                            lhsT=yT_sb[j][:, ti * P:(ti + 1) * P],
                            rhs=wp_sb[:, j * C + co * QB:j * C + (co + 1) * QB],
                            start=(j == 0), stop=(j == CL // P - 1))
                    return mm
                def evict():
                    osb = opool.tile([P, QB], bf, tag="o", name="osb")
                    if split and USE_LATE:
                        # final group: two 256-wide halves on separate engines
                        # with pipelined DMAs, to shorten the post-last-matmul
                        # eviction+DMA chain that is the kernel tail.
                        hq = QB // 2
                        nc.vector.tensor_copy(out=osb[:, 0:hq],
                                              in_=ps[0][:, 0:hq])
                        nc.sync.dma_start(
                            out=out_d.ap()[ti][:, co * QB:co * QB + hq],
                            in_=osb[:, 0:hq])
                        nc.scalar.copy(out=osb[:, hq:QB], in_=ps[0][:, hq:QB])
                        nc.sync.dma_start(
                            out=out_d.ap()[ti][:, co * QB + hq:(co + 1) * QB],
                            in_=osb[:, hq:QB])
                        return
                    if late and USE_LATE:
                        nc.scalar.copy(out=osb[:], in_=ps[0][:])
                    else:
                        nc.vector.tensor_copy(out=osb[:], in_=ps[0][:])
                    nc.sync.dma_start(
                        out=out_d.ap()[ti][:, co * QB:(co + 1) * QB],
                        in_=osb[:])
                return [mk(j) for j in range(CL // P)], evict

            # ---- filler machinery ----
            class Filler:
                def __init__(self):
                    self.q = deque()   # (label, mms list, evict)
                    self.total = 0
                def push(self, label, group):
                    mms, evict = group
                    self.q.append((label, list(mms), evict))
                    self.total += len(mms)
                def _step(self):
                    label, mms, evict = self.q[0]
                    mms.pop(0)()
                    self.total -= 1
                    if not mms:
                        evict()
                        self.q.popleft()
                def pull(self, n):
                    for _ in range(n):
                        if not self.q:
                            return
                        self._step()
                def drain_until(self, label):
                    while self.q and any(l == label for l, _, _ in self.q):
                        self._step()
                def mms_through(self, labels):
                    n, upto = 0, 0
                    for i, (l, mms, _) in enumerate(self.q):
                        n += len(mms)
                        if l in labels:
                            upto = n
                    return upto
                def drain_all(self):
                    while self.q:
                        self._step()
                def pull_whole_groups(self, n_mms):
                    done = 0
                    while self.q and done < n_mms:
                        done += len(self.q[0][1])
                        while self.q and self.q[0][1]:
                            self._step()

            filler = Filler()

            # ---- prologue: QKV for token chunk 0 ----
            for hb in range(NP):
                mms, evict = k_group(hb, 0)
                for mm in mms:
                    mm()
                evict()
            for ti in range(NQB):
                mms, evict = v_group(ti)
                for mm in mms:
                    mm()
                evict()
            for hb in range(NP):
                mms, evict = q_group(hb, 0)
                for mm in mms:
                    mm()
                evict()

            # queue remaining QKV work as filler
            for tch in range(1, NQB):
                for hb in range(NP):
                    filler.push(("K", tch), k_group(hb, tch))
                for hb in range(NP):
                    filler.push(("Q", tch), q_group(hb, tch))
                for ti in range(tch * 4, tch * 4 + 4):
                    filler.push(("V", tch), v_group(ti))

            # ---- attention chunk pipeline ----
            total_chunks = NP * sum(4 * (qb + 1) for qb in range(NQB))  # 160
            chunks_done = 0
            pending_pv = deque()

            class Blk:
                def __init__(self, hb, qb):
                    self.hb, self.qb = hb, qb
                    self.nkt = 4 * (qb + 1)
                    self.ps = None  # [even psum, odd psum]

            def emit_st_chunk(blk, kt):
                m = kt - 4 * blk.qb
                q0 = max(0, m * P)
                ps = ps_s.tile([P, 2 * QB], f32, tag="s", name="pss")
                for half in range(2):
                    base = half * D
                    nc.tensor.matmul(
                        ps[:, half * QB + q0:(half + 1) * QB],
                        lhsT=kT_sb[blk.hb][base:base + D, kt * P:(kt + 1) * P],
                        rhs=qT_sb[blk.hb][base:base + D,
                                          blk.qb * QB + q0:(blk.qb + 1) * QB],
                        start=True, stop=True)
                ch = cpool.tile([P, 2 * QB], bf, tag="ch", name="ch")
                if q0 == 0:
                    nc.scalar.activation(
                        out=ch[:], in_=ps[:],
                        func=mybir.ActivationFunctionType.Exp,
                        scale=float(1.0 / np.sqrt(D)))
                else:
                    # one ACT instruction for both halves via a 3D AP
                    ch3 = ch[:].rearrange("p (h q) -> p h q", q=QB)
                    ps3 = ps[:].rearrange("p (h q) -> p h q", q=QB)
                    nc.scalar.activation(
                        out=ch3[:, :, q0:QB], in_=ps3[:, :, q0:QB],
                        func=mybir.ActivationFunctionType.Exp,
                        scale=float(1.0 / np.sqrt(D)))
                if m >= 0:
                    for half in range(2):
                        dch = ch[:, half * QB + m * P:half * QB + (m + 1) * P]
                        nc.vector.tensor_tensor(
                            out=dch, in0=dch, in1=umask[:],
                            op=mybir.AluOpType.mult)
                return ch, q0

            def emit_pv_chunk(blk, kt, ch, q0):
                if blk.ps is None:
                    blk.ps = [ps_y.tile([D + 1, QB], f32, tag="y", name="psy0"),
                              ps_y.tile([D + 1, QB], f32, tag="y", name="psy1")]
                for he in range(2):
                    h = 2 * blk.hb + he
                    nc.tensor.matmul(
                        blk.ps[he][:, q0:QB],
                        lhsT=v65_sb[kt][:, h * (D + 1):(h + 1) * (D + 1)],
                        rhs=ch[:, he * QB + q0:(he + 1) * QB],
                        start=(kt == 0), stop=(kt == blk.nkt - 1))

            def emit_normalize(blk):
                # bf16 throughout: halves DVE stream cost (2x perf modes);
                # denominators are O(1e2-1e3) and y O(1), well within bf16.
                ytmp, rrow, rb = [], [], []
                for he in range(2):
                    ytmp.append(small.tile([D + 1, QB], bf, tag="ytmp",
                                           name="ytmp"))
                    nc.vector.tensor_copy(out=ytmp[he][:],
                                          in_=blk.ps[he][:, 0:QB])
                srow = []
                for he in range(2):
                    # reciprocal_approx_fast mis-lowers on HW when its input
                    # AP sits at base partition 64 — stage the denominator
                    # row through a partition-0 tile first. (fp32: the recip
                    # seed depends on fp32 bit layout.)
                    srow.append(small.tile([1, QB], f32, tag="srow",
                                           name="srow"))
                    nc.vector.tensor_copy(out=srow[he][:],
                                          in_=ytmp[he][D:D + 1, :])
                for he in range(2):
                    rrow.append(small.tile([1, QB], f32, tag="rrow",
                                           name="rrow"))
                    nc.vector.reciprocal_approx_fast(
                        rrow[he][:], srow[he][:])
                    rrowb = small.tile([1, QB], bf, tag="rrowb", name="rrowb")
                    nc.vector.tensor_copy(out=rrowb[:], in_=rrow[he][:])
                    rb.append(small.tile([D, QB], bf, tag="rb", name="rb"))
                    nc.gpsimd.partition_broadcast(rb[he][:], rrowb[:])
                for he in range(2):
                    nc.vector.tensor_tensor(
                        out=yT_sb[blk.hb][he * D:(he + 1) * D,
                                          blk.qb * QB:(blk.qb + 1) * QB],
                        in0=ytmp[he][0:D, :],
                        in1=rb[he][:],
                        op=mybir.AluOpType.mult)

            def pop_pv():
                blk, kt, ch, q0, last = pending_pv.popleft()
                emit_pv_chunk(blk, kt, ch, q0)
                if last:
                    emit_normalize(blk)
                    if blk.hb == NP - 1:
                        qb = blk.qb
                        for ti in range(qb * 4, qb * 4 + 4):
                            for co in range(C // QB):
                                filler.push(
                                    ("proj", qb),
                                    proj_group(ti, co,
                                               late=(2 if qb == 3 else
                                                     1 if qb == 2 else 0),
                                               split=False))

            for qb in range(NQB):
                if qb >= 1:
                    filler.drain_until(("Q", qb))
                    filler.drain_until(("V", qb))
                chunks_in_qb = NP * 4 * (qb + 1)
                chunks_in_qb_left = chunks_in_qb
                for hb in range(NP):
                    blk = Blk(hb, qb)
                    for kt in range(blk.nkt):
                        ch, q0 = emit_st_chunk(blk, kt)
                        pending_pv.append((blk, kt, ch, q0, kt == blk.nkt - 1))
                        chunks_done += 1
                        chunks_in_qb_left -= 1
                        if FINE_INTERLEAVE:
                            rem_chunks = total_chunks - chunks_done
                            if rem_chunks > 0:
                                avail = max(0, filler.total - TAIL_RESERVE_MMS)
                                want = -(-avail // rem_chunks)  # ceil
                                if qb < NQB - 1 and chunks_in_qb_left > 0:
                                    due = filler.mms_through(
                                        {("K", qb + 1), ("Q", qb + 1),
                                         ("V", qb + 1)})
                                    want = max(want,
                                               -(-due // chunks_in_qb_left))
                                while len(pending_pv) > LAG:
                                    pop_pv()
                                filler.pull(min(want, 8))
                            while len(pending_pv) > LAG:
                                pop_pv()
                    if not FINE_INTERLEAVE:
                        while pending_pv:
                            pop_pv()
                        nblk = chunks_done // 10 + 1
                        avail = max(0, filler.total - TAIL_RESERVE_MMS)
                        est = max(8, avail * blk.nkt // max(1, total_chunks - chunks_done))
                        filler.pull_whole_groups(est)
            while pending_pv:
                pop_pv()
                filler.pull(2)
            filler.drain_all()
            if DBG:
                def dump(dst, tiles):
                    for i, tsb in enumerate(tiles):
                        sh = [tsb.shape[0], tsb.shape[1]]
                        tmp = dpool.tile(sh, f32, tag="d", name="dtmp")
                        nc.vector.tensor_copy(out=tmp[:], in_=tsb[:])
                        nc.sync.dma_start(out=dst.ap()[i], in_=tmp[:])
                dump(dbg_qT, qT_sb)
                dump(dbg_kT, kT_sb)
                dump(dbg_v, v65_sb)
                dump(dbg_yT, yT_sb)

    nc.compile()
    return nc


def _get_nc():
    global _BUILT
    if _BUILT is None:
        _BUILT = _build()
    return _BUILT


def _shard_inputs(x, w_attn, b_attn, w_proj):
    in_maps = []
    for c in range(NCORES):
        b, hh = divmod(c, 2)
        hoff = hh * CL
        # xT[tch][p, kc*512+t]
        xT = np.ascontiguousarray(
            x[b].T.reshape(NCT, P, NQB, QB).transpose(2, 1, 0, 3)
            .reshape(NQB, P, NCT * QB)
        ).astype(BF16)

        def wpack_hb(w):  # [C, CL] -> [P, hb*1024 + kc*128 + d]
            # w[kc*128+p, hb*128+d] -> out[p, hb*(NCT*128) + kc*128 + d]
            return np.ascontiguousarray(
                w.reshape(NCT, P, NP, P).transpose(1, 2, 0, 3)
                .reshape(P, NCT * CL)
            ).astype(BF16)

        def wpack(w):  # [C, CL] -> [P, kc*CL + oc]
            return np.ascontiguousarray(
                w.reshape(NCT, P, CL).transpose(1, 0, 2).reshape(P, NCT * CL)
            ).astype(BF16)

        wqkQ = wpack_hb(w_attn[:, hoff:hoff + CL])
        wqkK = wpack_hb(w_attn[:, C + hoff:C + hoff + CL])
        wv = wpack(w_attn[:, 2 * C + hoff:2 * C + hoff + CL])
        bqk = np.ascontiguousarray(
            np.concatenate(
                [b_attn[hoff:hoff + CL], b_attn[C + hoff:C + hoff + CL]]
            ).astype(np.float32).reshape((2 * CL) // P, P).T
        )
        wp = np.ascontiguousarray(
            w_proj[hoff:hoff + CL].reshape(CL // P, P, C)
            .transpose(1, 0, 2).reshape(P, (CL // P) * C)
        ).astype(BF16)
        in_maps.append(
            {"xT": xT, "wqkQ": wqkQ, "wqkK": wqkK, "wv": wv,
             "bqk": bqk, "wp": wp}
        )
    return in_maps


def _run(in_maps, trace=False):
    from concourse.bass_utils import run_bass_kernel_spmd

    nc = _get_nc()
    return run_bass_kernel_spmd(
        nc, in_maps, core_ids=list(range(NCORES)), trace=trace
    )


def kernel(x, w_attn, b_attn, w_proj, b_proj):
    x = np.asarray(x, dtype=np.float32)
    w_attn = np.asarray(w_attn, dtype=np.float32)
    b_attn = np.asarray(b_attn, dtype=np.float32)
    w_proj = np.asarray(w_proj, dtype=np.float32)
    b_proj = np.asarray(b_proj, dtype=np.float32)

    in_maps = _shard_inputs(x, w_attn, b_attn, w_proj)
    res = _run(in_maps)
    parts = [
        res.results[c]["out"].reshape(T, C).astype(np.float32)
        for c in range(NCORES)
    ]
    # v-bias commutes through the attention average (sum_k p_k = 1), so it
    # lands as an extra output bias: b_eff = b_proj + b_v @ w_proj.
    b_eff = b_proj + b_attn[2 * C:3 * C] @ w_proj
    out = np.stack(
        [parts[2 * b] + parts[2 * b + 1] + b_eff for b in range(B)]
    ).astype(np.float32)
    return out



# revision 30
# speedup vs baseline: 1.0002x; 1.0002x over previous
"""Causal self-attention (B=4, T=2048, C=1024, H=16, D=64) on 8 TRN2 NeuronCores.

Sharding: core c handles batch b = c//2 and head-half hh = c%2 (8 of 16 heads).
Each core computes its partial c_proj output [T, C] in bf16; the host sums the
two partials per batch and adds b_proj + b_v @ w_proj (the v-bias commutes
through the attention average since softmax weights sum to 1).

v3: single dense PE instruction stream instead of serial phases.
  - qb-major attention with a chunk-granular software pipeline:
    S^T chunk (2 matmuls, head pair) -> exp (ScalarE) -> PV lagging LAG chunks.
  - Head-pair row-packing: even head uses PE rows 0-63, odd head rows 64-127
    (auto tile_position) so the two K=64 S^T matmuls run concurrently on HW.
  - QKV projection / c_proj groups interleaved between attention chunks as
    filler so PE stays busy while ScalarE exponentiates.
  - Softmax denominator rides the PV matmul as a ones-column in the V tile;
    normalize is a lean bf16 DVE chain (copy/recip/bcast/mult), GpSimd does
    the partition broadcast.
  - Diagonal chunks exp both head-halves in ONE ScalarE instruction (3D AP).
  - wqkQ/wqkK packed head-pair-major so the prologue consumes the DMA stream
    sequentially; first matmul starts as soon as ~160KB has landed. PE
    warm-up matmuls run during the initial DMA latency window (HAM ramp).
  - bf16 output (halved eviction + DMA-out cost); host accumulates in f32.
"""

import sys
from collections import deque

for _p in ("/opt/trn_rl_repo", "/root/.axon_site"):
    if _p not in sys.path:
        sys.path.append(_p)

import numpy as np
import ml_dtypes

BF16 = ml_dtypes.bfloat16

B, T, C, H = 4, 2048, 1024, 16
D = C // H          # 64
NCORES = 8
HL = H // 2         # 8 local heads
NP = HL // 2        # 4 head pairs
CL = HL * D         # 512 local qkv channels
P = 128
TT = T // P         # 16 token tiles
QB = 512            # query block
NQB = T // QB       # 4
NCT = C // P        # 8 contraction tiles

LAG = 4
FINE_INTERLEAVE = True   # pull filler MMs between chunks (vs whole groups at block ends)
USE_LATE = True          # scalar-copy evicts + ps_s borrowing for tail proj groups
TAIL_RESERVE_MMS = 48
NCHUNK_BUFS = 8

_BUILT = None


def _build():
    import concourse.mybir as mybir
    import concourse.tile as tile
    from concourse import bacc
    from concourse.masks import make_upper_triangular

    bf = mybir.dt.bfloat16
    f32 = mybir.dt.float32

    nc = bacc.Bacc("TRN2", target_bir_lowering=False, debug=False,
                   num_devices=NCORES)

    # xT[tch][p, kc*512 + t]: token chunk tch, contraction block kc
    xT_d = nc.dram_tensor("xT", [NQB, P, NCT * QB], bf, kind="ExternalInput")
    # wqk*[p, hb*1024 + kc*128 + d]: head-pair major so the prologue's
    # hb-ordered K groups consume the DMA stream sequentially
    wqkQ_d = nc.dram_tensor("wqkQ", [P, NCT * CL], bf, kind="ExternalInput")
    wqkK_d = nc.dram_tensor("wqkK", [P, NCT * CL], bf, kind="ExternalInput")
    # wv[p, kc*512 + oc]: contraction block kc, local out channel oc
    wv_d = nc.dram_tensor("wv", [P, NCT * CL], bf, kind="ExternalInput")
    bqk_d = nc.dram_tensor("bqk", [P, (2 * CL) // P], f32, kind="ExternalInput")
    # wp[p, j*1024 + oc]
    wp_d = nc.dram_tensor("wp", [P, (CL // P) * C], bf, kind="ExternalInput")
    out_d = nc.dram_tensor("out", [TT, P, C], bf, kind="ExternalOutput")
    import os
    DBG = bool(os.environ.get("K2_DEBUG"))
    if DBG:
        dbg_qT = nc.dram_tensor("dbg_qT", [NP, P, T], f32, kind="ExternalOutput")
        dbg_kT = nc.dram_tensor("dbg_kT", [NP, P, T], f32, kind="ExternalOutput")
        dbg_v = nc.dram_tensor("dbg_v", [TT, P, HL * (D + 1)], f32,
                               kind="ExternalOutput")
        dbg_yT = nc.dram_tensor("dbg_yT", [CL // P, P, T], f32,
                                kind="ExternalOutput")

    with tile.TileContext(nc) as tc:
        with (
            tc.tile_pool(name="const", bufs=1) as const,
            tc.tile_pool(name="weights", bufs=1) as wpool,
            tc.tile_pool(name="acts", bufs=1) as apool,
            tc.tile_pool(name="chunks", bufs=NCHUNK_BUFS) as cpool,
            tc.tile_pool(name="outsb", bufs=6) as opool,
            tc.tile_pool(name="small", bufs=6) as small,
            tc.tile_pool(name="dbgpool", bufs=2) as dpool,
            tc.tile_pool(name="ps_s", bufs=2, space="PSUM") as ps_s,
            tc.tile_pool(name="ps_y", bufs=2, space="PSUM") as ps_y,
            tc.tile_pool(name="ps_mm", bufs=2, space="PSUM") as ps_mm,
        ):
            # ---- SBUF tensors ----
            wqkQ_sb = wpool.tile([P, NCT * CL], bf, tag="wqkQ", name="wqkQ")
            wqkK_sb = wpool.tile([P, NCT * CL], bf, tag="wqkK", name="wqkK")
            wv_sb = wpool.tile([P, NCT * CL], bf, tag="wv", name="wv")
            wp_sb = wpool.tile([P, (CL // P) * C], bf, tag="wp", name="wp")
            xT_sb = [apool.tile([P, NCT * QB], bf, tag=f"xTt{t}", name=f"xTt{t}")
                     for t in range(NQB)]
            qT_sb = [apool.tile([P, T], bf, tag=f"qT{b}", name=f"qT{b}") for b in range(NP)]
            kT_sb = [apool.tile([P, T], bf, tag=f"kT{b}", name=f"kT{b}") for b in range(NP)]
            v65_sb = [apool.tile([P, HL * (D + 1)], bf, tag=f"v65{ti}", name=f"v65{ti}")
                      for ti in range(TT)]
            yT_sb = [apool.tile([P, T], bf, tag=f"yT{j}", name=f"yT{j}") for j in range(CL // P)]

            # ---- DMA emission (ordered for earliest compute start) ----
            # wqkK is hb-major: cols [hb*1024, (hb+1)*1024) cover one whole
            # k_group, consumed kc-sequentially. xT[0] feeds ALL four
            # K groups, so it streams at priority right after the first
            # small slices; later-hb weight blocks arrive during group 0.
            bqk_sb = const.tile([P, (2 * CL) // P], f32, tag="bqk")
            nc.sync.dma_start(out=wqkK_sb[:, 0:128], in_=wqkK_d.ap()[:, 0:128])
            nc.sync.dma_start(out=xT_sb[0][:, 0:512], in_=xT_d.ap()[0][:, 0:512])
            nc.sync.dma_start(out=bqk_sb[:], in_=bqk_d.ap())
            nc.sync.dma_start(out=wqkK_sb[:, 128:1024],
                              in_=wqkK_d.ap()[:, 128:1024])
            nc.sync.dma_start(out=xT_sb[0][:, 512:1024],
                              in_=xT_d.ap()[0][:, 512:1024])
            nc.sync.dma_start(out=xT_sb[0][:, 1024:2048],
                              in_=xT_d.ap()[0][:, 1024:2048])
            nc.sync.dma_start(out=xT_sb[0][:, 2048:4096],
                              in_=xT_d.ap()[0][:, 2048:4096])
            nc.sync.dma_start(out=wqkK_sb[:, 1024:2048],
                              in_=wqkK_d.ap()[:, 1024:2048])
            nc.sync.dma_start(out=wqkK_sb[:, 2048:4096],
                              in_=wqkK_d.ap()[:, 2048:4096])

            # ---- constants / act-table preload (off the critical DMA path) ----
            warm = const.tile([1, 16], f32, tag="warm")
            nc.vector.memset(warm[:], 0.0)
            wexp = const.tile([1, 16], f32, tag="wexp")
            nc.scalar.activation(out=wexp[:], in_=warm[:],
                                 func=mybir.ActivationFunctionType.Exp)
            # PE warm-up during the initial DMA latency window: keeps the
            # HAM activity monitor busy so real matmuls start at full clock.
            warm_bf = const.tile([1, 64], bf, tag="warmbf")
            nc.vector.memset(warm_bf[:], 0.0)
            ps_warm = ps_mm.tile([P, QB], f32, tag="mm", name="warmps")
            for _ in range(12):
                nc.tensor.matmul(ps_warm[0:64, 0:64],
                                 lhsT=warm_bf[:, 0:64],
                                 rhs=warm_bf[:, 0:64],
                                 start=True, stop=True)
            umask = const.tile([P, P], bf, tag="umask")
            make_upper_triangular(nc, umask[:], val=1.0, diag=True)
            nc.sync.dma_start(out=wv_sb[:], in_=wv_d.ap())
            nc.sync.dma_start(out=wqkQ_sb[:], in_=wqkQ_d.ap())
            for tch in range(1, NQB):
                nc.sync.dma_start(out=xT_sb[tch][:], in_=xT_d.ap()[tch])
            nc.sync.dma_start(out=wp_sb[:], in_=wp_d.ap())

            # ---- group emitters (each: list of MM closures + evict) ----
            def k_group(hb, tch):
                ps = [None]
                def mk(kc):
                    def mm():
                        if ps[0] is None:
                            ps[0] = ps_mm.tile([P, QB], f32, tag="mm", name="mm")
                        nc.tensor.matmul(
                            ps[0][:],
                            lhsT=wqkK_sb[:, hb * (NCT * P) + kc * P:
                                            hb * (NCT * P) + (kc + 1) * P],
                            rhs=xT_sb[tch][:, kc * QB:(kc + 1) * QB],
                            start=(kc == 0), stop=(kc == NCT - 1))
                    return mm
                def evict():
                    nc.vector.tensor_scalar_add(
                        kT_sb[hb][:, tch * QB:(tch + 1) * QB],
                        ps[0][:], bqk_sb[:, NP + hb:NP + hb + 1])
                return [mk(kc) for kc in range(NCT)], evict

            def q_group(hb, tch):
                ps = [None]
                def mk(kc):
                    def mm():
                        if ps[0] is None:
                            ps[0] = ps_mm.tile([P, QB], f32, tag="mm", name="mm")
                        nc.tensor.matmul(
                            ps[0][:],
                            lhsT=wqkQ_sb[:, hb * (NCT * P) + kc * P:
                                            hb * (NCT * P) + (kc + 1) * P],
                            rhs=xT_sb[tch][:, kc * QB:(kc + 1) * QB],
                            start=(kc == 0), stop=(kc == NCT - 1))
                    return mm
                def evict():
                    nc.vector.tensor_scalar_add(
                        qT_sb[hb][:, tch * QB:(tch + 1) * QB],
                        ps[0][:], bqk_sb[:, hb:hb + 1])
                return [mk(kc) for kc in range(NCT)], evict

            def v_group(ti):
                ps = [None]
                def mk(kc):
                    def mm():
                        if ps[0] is None:
                            ps[0] = ps_mm.tile([P, QB], f32, tag="mm", name="mm")
                        nc.tensor.matmul(
                            ps[0][:],
                            lhsT=xT_sb[ti // 4][:, kc * QB + (ti % 4) * P:
                                                  kc * QB + (ti % 4 + 1) * P],
                            rhs=wv_sb[:, kc * CL:(kc + 1) * CL],
                            start=(kc == 0), stop=(kc == NCT - 1))
                    return mm
                def evict():
                    # v65 layout per head: cols 0..63 = v, col 64 = ones (the
                    # softmax denominator rides the PV matmul). v-bias is
                    # folded into the host-side output bias (sum_k p_k = 1).
                    v3 = v65_sb[ti][:].rearrange("p (h e) -> p h e", e=D + 1)
                    nc.vector.tensor_copy(
                        out=v3[:, :, 0:D],
                        in_=ps[0][:].rearrange("p (h e) -> p h e", e=D))
                    nc.vector.memset(v3[:, :, D:D + 1], 1.0)
                return [mk(kc) for kc in range(NCT)], evict

            def proj_group(ti, co, late=False, split=False):
                ps = [None]
                def mk(j):
                    def mm():
                        if ps[0] is None:
                            if USE_LATE and late == 2 and (ti * 2 + co) % 2 == 0:
                                big = ps_s.tile([P, 2 * QB], f32, tag="s",
                                                name="pss")
                                ps[0] = big[:, 0:QB]
                            else:
                                ps[0] = ps_mm.tile([P, QB], f32, tag="mm",
                                                   name="mm")
                        nc.tensor.matmul(
                            ps[0][:],
                            lhsT=yT_sb[j][:, ti * P:(ti + 1) * P],
                            rhs=wp_sb[:, j * C + co * QB:j * C + (co + 1) * QB],
                            start=(j == 0), stop=(j == CL // P - 1))
                    return mm
                def evict():
                    osb = opool.tile([P, QB], bf, tag="o", name="osb")
                    if split and USE_LATE:
                        # final group: two 256-wide halves on separate engines
                        # with pipelined DMAs, to shorten the post-last-matmul
                        # eviction+DMA chain that is the kernel tail.
                        hq = QB // 2
                        nc.vector.tensor_copy(out=osb[:, 0:hq],
                                              in_=ps[0][:, 0:hq])
                        nc.sync.dma_start(
                            out=out_d.ap()[ti][:, co * QB:co * QB + hq],
                            in_=osb[:, 0:hq])
                        nc.scalar.copy(out=osb[:, hq:QB], in_=ps[0][:, hq:QB])
                        nc.sync.dma_start(
                            out=out_d.ap()[ti][:, co * QB + hq:(co + 1) * QB],
                            in_=osb[:, hq:QB])
                        return
                    if late and USE_LATE:
                        nc.scalar.copy(out=osb[:], in_=ps[0][:])
                    else:
                        nc.vector.tensor_copy(out=osb[:], in_=ps[0][:])
                    nc.sync.dma_start(
                        out=out_d.ap()[ti][:, co * QB:(co + 1) * QB],
                        in_=osb[:])
                return [mk(j) for j in range(CL // P)], evict

            # ---- filler machinery ----
            class Filler:
                def __init__(self):
                    self.q = deque()   # (label, mms list, evict)
                    self.total = 0
                def push(self, label, group):
                    mms, evict = group
                    self.q.append((label, list(mms), evict))
                    self.total += len(mms)
                def _step(self):
                    label, mms, evict = self.q[0]
                    mms.pop(0)()
                    self.total -= 1
                    if not mms:
                        evict()
                        self.q.popleft()
                def pull(self, n):
                    for _ in range(n):
                        if not self.q:
                            return
                        self._step()
                def drain_until(self, label):
                    while self.q and any(l == label for l, _, _ in self.q):
                        self._step()
                def mms_through(self, labels):
                    n, upto = 0, 0
                    for i, (l, mms, _) in enumerate(self.q):
                        n += len(mms)
                        if l in labels:
                            upto = n
                    return upto
                def drain_all(self):
                    while self.q:
                        self._step()
                def pull_whole_groups(self, n_mms):
                    done = 0
                    while self.q and done < n_mms:
                        done += len(self.q[0][1])
                        while self.q and self.q[0][1]:
                            self._step()

            filler = Filler()

            # ---- prologue: QKV for token chunk 0 ----
            for hb in range(NP):
                mms, evict = k_group(hb, 0)
                for mm in mms:
                    mm()
                evict()
            for ti in range(NQB):
                mms, evict = v_group(ti)
                for mm in mms:
                    mm()
                evict()
            for hb in range(NP):
                mms, evict = q_group(hb, 0)
                for mm in mms:
                    mm()
                evict()

            # queue remaining QKV work as filler
            for tch in range(1, NQB):
                for hb in range(NP):
                    filler.push(("K", tch), k_group(hb, tch))
                for hb in range(NP):
                    filler.push(("Q", tch), q_group(hb, tch))
                for ti in range(tch * 4, tch * 4 + 4):
                    filler.push(("V", tch), v_group(ti))

            # ---- attention chunk pipeline ----
            total_chunks = NP * sum(4 * (qb + 1) for qb in range(NQB))  # 160
            chunks_done = 0
            pending_pv = deque()

            class Blk:
                def __init__(self, hb, qb):
                    self.hb, self.qb = hb, qb
                    self.nkt = 4 * (qb + 1)
                    self.ps = None  # [even psum, odd psum]

            def emit_st_chunk(blk, kt):
                m = kt - 4 * blk.qb
                q0 = max(0, m * P)
                ps = ps_s.tile([P, 2 * QB], f32, tag="s", name="pss")
                for half in range(2):
                    base = half * D
                    nc.tensor.matmul(
                        ps[:, half * QB + q0:(half + 1) * QB],
                        lhsT=kT_sb[blk.hb][base:base + D, kt * P:(kt + 1) * P],
                        rhs=qT_sb[blk.hb][base:base + D,
                                          blk.qb * QB + q0:(blk.qb + 1) * QB],
                        start=True, stop=True)
                ch = cpool.tile([P, 2 * QB], bf, tag="ch", name="ch")
                if q0 == 0:
                    nc.scalar.activation(
                        out=ch[:], in_=ps[:],
                        func=mybir.ActivationFunctionType.Exp,
                        scale=float(1.0 / np.sqrt(D)))
                else:
                    # one ACT instruction for both halves via a 3D AP
                    ch3 = ch[:].rearrange("p (h q) -> p h q", q=QB)
                    ps3 = ps[:].rearrange("p (h q) -> p h q", q=QB)
                    nc.scalar.activation(
                        out=ch3[:, :, q0:QB], in_=ps3[:, :, q0:QB],
                        func=mybir.ActivationFunctionType.Exp,
                        scale=float(1.0 / np.sqrt(D)))
                if m >= 0:
                    for half in range(2):
                        dch = ch[:, half * QB + m * P:half * QB + (m + 1) * P]
                        nc.vector.tensor_tensor(
                            out=dch, in0=dch, in1=umask[:],
                            op=mybir.AluOpType.mult)
                return ch, q0

            def emit_pv_chunk(blk, kt, ch, q0):
                if blk.ps is None:
                    blk.ps = [ps_y.tile([D + 1, QB], f32, tag="y", name="psy0"),
                              ps_y.tile([D + 1, QB], f32, tag="y", name="psy1")]
                for he in range(2):
                    h = 2 * blk.hb + he
                    nc.tensor.matmul(
                        blk.ps[he][:, q0:QB],
                        lhsT=v65_sb[kt][:, h * (D + 1):(h + 1) * (D + 1)],
                        rhs=ch[:, he * QB + q0:(he + 1) * QB],
                        start=(kt == 0), stop=(kt == blk.nkt - 1))

            def emit_normalize(blk):
                # bf16 throughout: halves DVE stream cost (2x perf modes);
                # denominators are O(1e2-1e3) and y O(1), well within bf16.
                ytmp, rrow, rb = [], [], []
                for he in range(2):
                    ytmp.append(small.tile([D + 1, QB], bf, tag="ytmp",
                                           name="ytmp"))
                    nc.vector.tensor_copy(out=ytmp[he][:],
                                          in_=blk.ps[he][:, 0:QB])
                srow = []
                for he in range(2):
                    # reciprocal_approx_fast mis-lowers on HW when its input
                    # AP sits at base partition 64 — stage the denominator
                    # row through a partition-0 tile first. (fp32: the recip
                    # seed depends on fp32 bit layout.)
                    srow.append(small.tile([1, QB], f32, tag="srow",
                                           name="srow"))
                    nc.vector.tensor_copy(out=srow[he][:],
                                          in_=ytmp[he][D:D + 1, :])
                for he in range(2):
                    rrow.append(small.tile([1, QB], f32, tag="rrow",
                                           name="rrow"))
                    nc.vector.reciprocal_approx_fast(
                        rrow[he][:], srow[he][:])
                    rrowb = small.tile([1, QB], bf, tag="rrowb", name="rrowb")
                    nc.vector.tensor_copy(out=rrowb[:], in_=rrow[he][:])
                    rb.append(small.tile([D, QB], bf, tag="rb", name="rb"))
                    nc.gpsimd.partition_broadcast(rb[he][:], rrowb[:])
                for he in range(2):
                    nc.vector.tensor_tensor(
                        out=yT_sb[blk.hb][he * D:(he + 1) * D,
                                          blk.qb * QB:(blk.qb + 1) * QB],
                        in0=ytmp[he][0:D, :],
                        in1=rb[he][:],
                        op=mybir.AluOpType.mult)

            def pop_pv():
                blk, kt, ch, q0, last = pending_pv.popleft()
                emit_pv_chunk(blk, kt, ch, q0)
                if last:
                    emit_normalize(blk)
                    if blk.hb == NP - 1:
                        qb = blk.qb
                        for ti in range(qb * 4, qb * 4 + 4):
                            for co in range(C // QB):
                                filler.push(
                                    ("proj", qb),
                                    proj_group(ti, co,
                                               late=(2 if qb == 3 else
                                                     1 if qb == 2 else 0),
                                               split=False))

            for qb in range(NQB):
                if qb >= 1:
                    filler.drain_until(("Q", qb))
                    filler.drain_until(("V", qb))
                chunks_in_qb = NP * 4 * (qb + 1)
                chunks_in_qb_left = chunks_in_qb
                for hb in range(NP):
                    blk = Blk(hb, qb)
                    for kt in range(blk.nkt):
                        ch, q0 = emit_st_chunk(blk, kt)
                        pending_pv.append((blk, kt, ch, q0, kt == blk.nkt - 1))
                        chunks_done += 1
                        chunks_in_qb_left -= 1
                        if FINE_INTERLEAVE:
                            rem_chunks = total_chunks - chunks_done
                            if rem_chunks > 0:
                                avail = max(0, filler.total - TAIL_RESERVE_MMS)
                                want = -(-avail // rem_chunks)  # ceil
                                if qb < NQB - 1 and chunks_in_qb_left > 0:
                                    due = filler.mms_through(
                                        {("K", qb + 1), ("Q", qb + 1),
                                         ("V", qb + 1)})
                                    want = max(want,
                                               -(-due // chunks_in_qb_left))
                                while len(pending_pv) > LAG:
                                    pop_pv()
                                filler.pull(min(want, 8))
                            while len(pending_pv) > LAG:
                                pop_pv()
                    if not FINE_INTERLEAVE:
                        while pending_pv:
                            pop_pv()
                        nblk = chunks_done // 10 + 1
                        avail = max(0, filler.total - TAIL_RESERVE_MMS)
                        est = max(8, avail * blk.nkt // max(1, total_chunks - chunks_done))
                        filler.pull_whole_groups(est)
            while pending_pv:
                pop_pv()
                filler.pull(2)
            filler.drain_all()
            if DBG:
                def dump(dst, tiles):
                    for i, tsb in enumerate(tiles):
                        sh = [tsb.shape[0], tsb.shape[1]]
                        tmp = dpool.tile(sh, f32, tag="d", name="dtmp")
                        nc.vector.tensor_copy(out=tmp[:], in_=tsb[:])
                        nc.sync.dma_start(out=dst.ap()[i], in_=tmp[:])
                dump(dbg_qT, qT_sb)
                dump(dbg_kT, kT_sb)
                dump(dbg_v, v65_sb)
                dump(dbg_yT, yT_sb)

    nc.compile()
    return nc


def _get_nc():
    global _BUILT
    if _BUILT is None:
        _BUILT = _build()
    return _BUILT


def _shard_inputs(x, w_attn, b_attn, w_proj):
    in_maps = []
    for c in range(NCORES):
        b, hh = divmod(c, 2)
        hoff = hh * CL
        # xT[tch][p, kc*512+t]
        xT = np.ascontiguousarray(
            x[b].T.reshape(NCT, P, NQB, QB).transpose(2, 1, 0, 3)
            .reshape(NQB, P, NCT * QB)
        ).astype(BF16)

        def wpack_hb(w):  # [C, CL] -> [P, hb*1024 + kc*128 + d]
            # w[kc*128+p, hb*128+d] -> out[p, hb*(NCT*128) + kc*128 + d]
            return np.ascontiguousarray(
                w.reshape(NCT, P, NP, P).transpose(1, 2, 0, 3)
                .reshape(P, NCT * CL)
            ).astype(BF16)

        def wpack(w):  # [C, CL] -> [P, kc*CL + oc]
            return np.ascontiguousarray(
                w.reshape(NCT, P, CL).transpose(1, 0, 2).reshape(P, NCT * CL)
            ).astype(BF16)

        wqkQ = wpack_hb(w_attn[:, hoff:hoff + CL])
        wqkK = wpack_hb(w_attn[:, C + hoff:C + hoff + CL])
        wv = wpack(w_attn[:, 2 * C + hoff:2 * C + hoff + CL])
        bqk = np.ascontiguousarray(
            np.concatenate(
                [b_attn[hoff:hoff + CL], b_attn[C + hoff:C + hoff + CL]]
            ).astype(np.float32).reshape((2 * CL) // P, P).T
        )
        wp = np.ascontiguousarray(
            w_proj[hoff:hoff + CL].reshape(CL // P, P, C)
            .transpose(1, 0, 2).reshape(P, (CL // P) * C)
        ).astype(BF16)
        in_maps.append(
            {"xT": xT, "wqkQ": wqkQ, "wqkK": wqkK, "wv": wv,
             "bqk": bqk, "wp": wp}
        )
    return in_maps


def _run(in_maps, trace=False):
    from concourse.bass_utils import run_bass_kernel_spmd

    nc = _get_nc()
    return run_bass_kernel_spmd(
        nc, in_maps, core_ids=list(range(NCORES)), trace=trace
    )


def kernel(x, w_attn, b_attn, w_proj, b_proj):
    x = np.asarray(x, dtype=np.float32)
    w_attn = np.asarray(w_attn, dtype=np.float32)
    b_attn = np.asarray(b_attn, dtype=np.float32)
    w_proj = np.asarray(w_proj, dtype=np.float32)
    b_proj = np.asarray(b_proj, dtype=np.float32)

    in_maps = _shard_inputs(x, w_attn, b_attn, w_proj)
    res = _run(in_maps)
    parts = [
        res.results[c]["out"].reshape(T, C).astype(np.float32)
        for c in range(NCORES)
    ]
    # v-bias commutes through the attention average (sum_k p_k = 1), so it
    # lands as an extra output bias: b_eff = b_proj + b_v @ w_proj.
    b_eff = b_proj + b_attn[2 * C:3 * C] @ w_proj
    out = np.stack(
        [parts[2 * b] + parts[2 * b + 1] + b_eff for b in range(B)]
    ).astype(np.float32)
    return out



# revision 47
# speedup vs baseline: 1.0102x; 1.0100x over previous
"""Causal self-attention (B=4, T=2048, C=1024, H=16, D=64) on 8 TRN2 NeuronCores.

Sharding: core c handles batch b = c//2 and head-half hh = c%2 (8 of 16 heads).
Each core computes its partial c_proj output [T, C] in bf16; the host sums the
two partials per batch and adds b_proj + b_v @ w_proj (the v-bias commutes
through the attention average since softmax weights sum to 1).

v3: single dense PE instruction stream instead of serial phases.
  - qb-major attention with a chunk-granular software pipeline:
    S^T chunk (2 matmuls, head pair) -> exp (ScalarE) -> PV lagging LAG chunks.
  - Head-pair row-packing: even head uses PE rows 0-63, odd head rows 64-127
    (auto tile_position) so the two K=64 S^T matmuls run concurrently on HW.
  - QKV projection / c_proj groups interleaved between attention chunks as
    filler so PE stays busy while ScalarE exponentiates.
  - Softmax denominator rides the PV matmul as a ones-column in the V tile;
    normalize is a lean bf16 DVE chain (copy/recip/bcast/mult), GpSimd does
    the partition broadcast.
  - Diagonal chunks exp both head-halves in ONE ScalarE instruction (3D AP).
  - Prologue K phase runs kc-outer across all four head-pairs (borrowing
    the idle ps_s banks as accumulators) so each arriving {wqkK, xT0} DMA
    block feeds 4 matmuls — compute matches the stream rate. PE warm-up
    matmuls run during the initial DMA latency window (HAM ramp).
  - bf16 output (halved eviction + DMA-out cost); host accumulates in f32.
"""

import sys
from collections import deque

for _p in ("/opt/trn_rl_repo", "/root/.axon_site"):
    if _p not in sys.path:
        sys.path.append(_p)

import numpy as np
import ml_dtypes

BF16 = ml_dtypes.bfloat16

B, T, C, H = 4, 2048, 1024, 16
D = C // H          # 64
NCORES = 8
HL = H // 2         # 8 local heads
NP = HL // 2        # 4 head pairs
CL = HL * D         # 512 local qkv channels
P = 128
TT = T // P         # 16 token tiles
QB = 512            # query block
NQB = T // QB       # 4
NCT = C // P        # 8 contraction tiles

LAG = 4
FINE_INTERLEAVE = True   # pull filler MMs between chunks (vs whole groups at block ends)
USE_LATE = True          # scalar-copy evicts + ps_s borrowing for tail proj groups
TAIL_RESERVE_MMS = 48
NCHUNK_BUFS = 8
WANT_CAP = 8             # max filler MMs pulled between consecutive chunks
DRAIN_PULL = 2           # filler MMs pulled per pop_pv in the final drain

_BUILT = None


def _build():
    import concourse.mybir as mybir
    import concourse.tile as tile
    from concourse import bacc
    from concourse.masks import make_upper_triangular

    bf = mybir.dt.bfloat16
    f32 = mybir.dt.float32

    nc = bacc.Bacc("TRN2", target_bir_lowering=False, debug=False,
                   num_devices=NCORES)

    # xT[tch][p, kc*512 + t]: token chunk tch, contraction block kc
    xT_d = nc.dram_tensor("xT", [NQB, P, NCT * QB], bf, kind="ExternalInput")
    # wqk*[p, kc*512 + oc]: contraction block kc, local out channel oc
    wqkQ_d = nc.dram_tensor("wqkQ", [P, NCT * CL], bf, kind="ExternalInput")
    wqkK_d = nc.dram_tensor("wqkK", [P, NCT * CL], bf, kind="ExternalInput")
    # wv[p, kc*512 + oc]: contraction block kc, local out channel oc
    wv_d = nc.dram_tensor("wv", [P, NCT * CL], bf, kind="ExternalInput")
    bqk_d = nc.dram_tensor("bqk", [P, (2 * CL) // P], f32, kind="ExternalInput")
    # wp[p, j*1024 + oc]
    wp_d = nc.dram_tensor("wp", [P, (CL // P) * C], bf, kind="ExternalInput")
    out_d = nc.dram_tensor("out", [TT, P, C], bf, kind="ExternalOutput")
    import os
    DBG = bool(os.environ.get("K2_DEBUG"))
    if DBG:
        dbg_qT = nc.dram_tensor("dbg_qT", [NP, P, T], f32, kind="ExternalOutput")
        dbg_kT = nc.dram_tensor("dbg_kT", [NP, P, T], f32, kind="ExternalOutput")
        dbg_v = nc.dram_tensor("dbg_v", [TT, P, HL * (D + 1)], f32,
                               kind="ExternalOutput")
        dbg_yT = nc.dram_tensor("dbg_yT", [CL // P, P, T], f32,
                                kind="ExternalOutput")

    with tile.TileContext(nc) as tc:
        with (
            tc.tile_pool(name="const", bufs=1) as const,
            tc.tile_pool(name="weights", bufs=1) as wpool,
            tc.tile_pool(name="acts", bufs=1) as apool,
            tc.tile_pool(name="chunks", bufs=NCHUNK_BUFS) as cpool,
            tc.tile_pool(name="outsb", bufs=6) as opool,
            tc.tile_pool(name="small", bufs=6) as small,
            tc.tile_pool(name="dbgpool", bufs=2) as dpool,
            tc.tile_pool(name="ps_s", bufs=2, space="PSUM") as ps_s,
            tc.tile_pool(name="ps_y", bufs=2, space="PSUM") as ps_y,
            tc.tile_pool(name="ps_mm", bufs=2, space="PSUM") as ps_mm,
        ):
            # ---- SBUF tensors ----
            wqkQ_sb = wpool.tile([P, NCT * CL], bf, tag="wqkQ", name="wqkQ")
            wqkK_sb = wpool.tile([P, NCT * CL], bf, tag="wqkK", name="wqkK")
            wv_sb = wpool.tile([P, NCT * CL], bf, tag="wv", name="wv")
            wp_sb = wpool.tile([P, (CL // P) * C], bf, tag="wp", name="wp")
            xT_sb = [apool.tile([P, NCT * QB], bf, tag=f"xTt{t}", name=f"xTt{t}")
                     for t in range(NQB)]
            qT_sb = [apool.tile([P, T], bf, tag=f"qT{b}", name=f"qT{b}") for b in range(NP)]
            kT_sb = [apool.tile([P, T], bf, tag=f"kT{b}", name=f"kT{b}") for b in range(NP)]
            v65_sb = [apool.tile([P, HL * (D + 1)], bf, tag=f"v65{ti}", name=f"v65{ti}")
                      for ti in range(TT)]
            yT_sb = [apool.tile([P, T], bf, tag=f"yT{j}", name=f"yT{j}") for j in range(CL // P)]

            # ---- DMA emission (ordered for earliest compute start) ----
            # The prologue K phase runs kc-outer over all four head-pairs,
            # so it consumes interleaved {wqkK, xT0} kc-blocks: each arriving
            # 512-col block feeds 4 matmuls (~852ns), matching the DMA
            # stream rate. wqkK is kc-major: cols [kc*512,(kc+1)*512) hold
            # all four head-pairs' weights for contraction tile kc.
            bqk_sb = const.tile([P, (2 * CL) // P], f32, tag="bqk")
            nc.sync.dma_start(out=wqkK_sb[:, 0:128], in_=wqkK_d.ap()[:, 0:128])
            nc.sync.dma_start(out=xT_sb[0][:, 0:512], in_=xT_d.ap()[0][:, 0:512])
            nc.sync.dma_start(out=wqkK_sb[:, 128:512],
                              in_=wqkK_d.ap()[:, 128:512])
            nc.sync.dma_start(out=wqkK_sb[:, 512:1536],
                              in_=wqkK_d.ap()[:, 512:1536])
            nc.sync.dma_start(out=xT_sb[0][:, 512:1536],
                              in_=xT_d.ap()[0][:, 512:1536])
            nc.sync.dma_start(out=wqkK_sb[:, 1536:2560],
                              in_=wqkK_d.ap()[:, 1536:2560])
            nc.sync.dma_start(out=xT_sb[0][:, 1536:2560],
                              in_=xT_d.ap()[0][:, 1536:2560])
            nc.sync.dma_start(out=wqkK_sb[:, 2560:3584],
                              in_=wqkK_d.ap()[:, 2560:3584])
            nc.sync.dma_start(out=xT_sb[0][:, 2560:3584],
                              in_=xT_d.ap()[0][:, 2560:3584])
            nc.sync.dma_start(out=wqkK_sb[:, 3584:4096],
                              in_=wqkK_d.ap()[:, 3584:4096])
            nc.sync.dma_start(out=xT_sb[0][:, 3584:4096],
                              in_=xT_d.ap()[0][:, 3584:4096])

            # ---- constants / act-table preload (off the critical DMA path) ----
            warm = const.tile([1, 16], f32, tag="warm")
            nc.vector.memset(warm[:], 0.0)
            wexp = const.tile([1, 16], f32, tag="wexp")
            nc.scalar.activation(out=wexp[:], in_=warm[:],
                                 func=mybir.ActivationFunctionType.Exp)
            # PE warm-up during the initial DMA latency window: keeps the
            # HAM activity monitor busy so real matmuls start at full clock.
            warm_bf = const.tile([1, 64], bf, tag="warmbf")
            nc.vector.memset(warm_bf[:], 0.0)
            ps_warm = ps_mm.tile([P, QB], f32, tag="mm", name="warmps")
            for _ in range(12):
                nc.tensor.matmul(ps_warm[0:64, 0:64],
                                 lhsT=warm_bf[:, 0:64],
                                 rhs=warm_bf[:, 0:64],
                                 start=True, stop=True)
            umask = const.tile([P, P], bf, tag="umask")
            make_upper_triangular(nc, umask[:], val=1.0, diag=True)
            nc.sync.dma_start(out=wv_sb[:, 0:1024], in_=wv_d.ap()[:, 0:1024])
            nc.sync.dma_start(out=wv_sb[:, 1024:2560],
                              in_=wv_d.ap()[:, 1024:2560])
            nc.sync.dma_start(out=wv_sb[:, 2560:4096],
                              in_=wv_d.ap()[:, 2560:4096])
            nc.sync.dma_start(out=bqk_sb[:], in_=bqk_d.ap())
            nc.sync.dma_start(out=wqkQ_sb[:], in_=wqkQ_d.ap())
            for tch in range(1, NQB):
                nc.sync.dma_start(out=xT_sb[tch][:], in_=xT_d.ap()[tch])
            nc.sync.dma_start(out=wp_sb[:], in_=wp_d.ap())

            # ---- group emitters (each: list of MM closures + evict) ----
            def k_group(hb, tch):
                ps = [None]
                def mk(kc):
                    def mm():
                        if ps[0] is None:
                            ps[0] = ps_mm.tile([P, QB], f32, tag="mm", name="mm")
                        nc.tensor.matmul(
                            ps[0][:],
                            lhsT=wqkK_sb[:, kc * CL + hb * P:
                                            kc * CL + (hb + 1) * P],
                            rhs=xT_sb[tch][:, kc * QB:(kc + 1) * QB],
                            start=(kc == 0), stop=(kc == NCT - 1))
                    return mm
                def evict():
                    # No k-bias: for a fixed query, (q+bq)·bk is constant
                    # across keys, so it cancels in softmax — only bq enters
                    # (via bq·k). Exact equivalence; plain copy eviction.
                    nc.vector.tensor_copy(
                        out=kT_sb[hb][:, tch * QB:(tch + 1) * QB],
                        in_=ps[0][:])
                return [mk(kc) for kc in range(NCT)], evict

            def q_group(hb, tch):
                ps = [None]
                def mk(kc):
                    def mm():
                        if ps[0] is None:
                            ps[0] = ps_mm.tile([P, QB], f32, tag="mm", name="mm")
                        nc.tensor.matmul(
                            ps[0][:],
                            lhsT=wqkQ_sb[:, kc * CL + hb * P:
                                            kc * CL + (hb + 1) * P],
                            rhs=xT_sb[tch][:, kc * QB:(kc + 1) * QB],
                            start=(kc == 0), stop=(kc == NCT - 1))
                    return mm
                def evict():
                    nc.vector.tensor_scalar_add(
                        qT_sb[hb][:, tch * QB:(tch + 1) * QB],
                        ps[0][:], bqk_sb[:, hb:hb + 1])
                return [mk(kc) for kc in range(NCT)], evict

            def v_group(ti):
                ps = [None]
                def mk(kc):
                    def mm():
                        if ps[0] is None:
                            ps[0] = ps_mm.tile([P, QB], f32, tag="mm", name="mm")
                        nc.tensor.matmul(
                            ps[0][:],
                            lhsT=xT_sb[ti // 4][:, kc * QB + (ti % 4) * P:
                                                  kc * QB + (ti % 4 + 1) * P],
                            rhs=wv_sb[:, kc * CL:(kc + 1) * CL],
                            start=(kc == 0), stop=(kc == NCT - 1))
                    return mm
                def evict():
                    # v65 layout per head: cols 0..63 = v, col 64 = ones (the
                    # softmax denominator rides the PV matmul). v-bias is
                    # folded into the host-side output bias (sum_k p_k = 1).
                    # Copy on ScalarE: keeps the forced-drain eviction chain
                    # off DVE (ps_mm reuse is gated by evictions there).
                    v3 = v65_sb[ti][:].rearrange("p (h e) -> p h e", e=D + 1)
                    nc.scalar.copy(
                        out=v3[:, :, 0:D],
                        in_=ps[0][:].rearrange("p (h e) -> p h e", e=D))
                    nc.vector.memset(v3[:, :, D:D + 1], 1.0)
                return [mk(kc) for kc in range(NCT)], evict

            def proj_group(ti, co, late=False, split=False):
                ps = [None]
                if split and USE_LATE:
                    # Final group: two 256-wide column-halves accumulated in
                    # DIFFERENT PSUM banks of one borrowed ps_s tile, so half
                    # A's evict+DMA overlaps half B's matmuls and only a
                    # 256-wide chain trails the last matmul (kernel tail).
                    hq = QB // 2
                    def evict_half(half, eng):
                        osb = opool.tile([P, hq], bf, tag="o", name="osb")
                        if eng == "act":
                            nc.scalar.copy(out=osb[:], in_=ps[0][half][:])
                        else:
                            nc.vector.tensor_copy(out=osb[:],
                                                  in_=ps[0][half][:])
                        nc.sync.dma_start(
                            out=out_d.ap()[ti][:, co * QB + half * hq:
                                               co * QB + (half + 1) * hq],
                            in_=osb[:])
                    def mk2(idx):
                        def mm():
                            if ps[0] is None:
                                # separate tiles from separate pools: Tile's
                                # dependency tracker is tile-granular, and
                                # the halves must not gate on one rotation.
                                bigA = ps_s.tile([P, 2 * QB], f32, tag="s",
                                                 name="pss")
                                psB = ps_mm.tile([P, QB], f32, tag="mm",
                                                 name="mm")
                                ps[0] = [bigA[:, 0:hq], psB[:, 0:hq]]
                            half, j = divmod(idx, CL // P)
                            nc.tensor.matmul(
                                ps[0][half][:],
                                lhsT=yT_sb[j][:, ti * P:(ti + 1) * P],
                                rhs=wp_sb[:, j * C + co * QB + half * hq:
                                           j * C + co * QB + half * hq + hq],
                                start=(j == 0), stop=(j == CL // P - 1))
                            if idx == CL // P - 1:
                                evict_half(0, "dve")
                        return mm
                    def evict():
                        evict_half(1, "act")
                    return [mk2(i) for i in range(2 * (CL // P))], evict
                def mk(j):
                    def mm():
                        if ps[0] is None:
                            if USE_LATE and late == 2 and (ti * 2 + co) % 2 == 0:
                                big = ps_s.tile([P, 2 * QB], f32, tag="s",
                                                name="pss")
                                ps[0] = big[:, 0:QB]
                            else:
                                ps[0] = ps_mm.tile([P, QB], f32, tag="mm",
                                                   name="mm")
                        nc.tensor.matmul(
                            ps[0][:],
                            lhsT=yT_sb[j][:, ti * P:(ti + 1) * P],
                            rhs=wp_sb[:, j * C + co * QB:j * C + (co + 1) * QB],
                            start=(j == 0), stop=(j == CL // P - 1))
                    return mm
                def evict():
                    osb = opool.tile([P, QB], bf, tag="o", name="osb")
                    if late and USE_LATE:
                        nc.scalar.copy(out=osb[:], in_=ps[0][:])
                    else:
                        nc.vector.tensor_copy(out=osb[:], in_=ps[0][:])
                    nc.sync.dma_start(
                        out=out_d.ap()[ti][:, co * QB:(co + 1) * QB],
                        in_=osb[:])
                return [mk(j) for j in range(CL // P)], evict

            # ---- filler machinery ----
            class Filler:
                def __init__(self):
                    self.q = deque()   # (label, mms list, evict)
                    self.total = 0
                def push(self, label, group):
                    mms, evict = group
                    self.q.append((label, list(mms), evict))
                    self.total += len(mms)
                def _step(self):
                    label, mms, evict = self.q[0]
                    mms.pop(0)()
                    self.total -= 1
                    if not mms:
                        evict()
                        self.q.popleft()
                def pull(self, n):
                    for _ in range(n):
                        if not self.q:
                            return
                        self._step()
                def drain_until(self, label):
                    while self.q and any(l == label for l, _, _ in self.q):
                        self._step()
                def mms_through(self, labels):
                    n, upto = 0, 0
                    for i, (l, mms, _) in enumerate(self.q):
                        n += len(mms)
                        if l in labels:
                            upto = n
                    return upto
                def drain_all(self):
                    while self.q:
                        self._step()
                def pull_whole_groups(self, n_mms):
                    done = 0
                    while self.q and done < n_mms:
                        done += len(self.q[0][1])
                        while self.q and self.q[0][1]:
                            self._step()

            filler = Filler()

            # ---- prologue: QKV for token chunk 0 ----
            # K phase kc-outer: all four head-pairs accumulate per kc block,
            # borrowing ps_s (idle until attention) as the four accumulators
            # (two [128,1024] tiles, one bank per head-pair half). Each
            # arriving DMA block is consumed 4x, matching the stream rate.
            kpsA = ps_s.tile([P, 2 * QB], f32, tag="s", name="pss")
            kpsB = ps_s.tile([P, 2 * QB], f32, tag="s", name="pss")
            kps = [kpsA[:, 0:QB], kpsA[:, QB:2 * QB],
                   kpsB[:, 0:QB], kpsB[:, QB:2 * QB]]
            for kc in range(NCT):
                for hb in range(NP):
                    nc.tensor.matmul(
                        kps[hb][:],
                        lhsT=wqkK_sb[:, kc * CL + hb * P:kc * CL + (hb + 1) * P],
                        rhs=xT_sb[0][:, kc * QB:(kc + 1) * QB],
                        start=(kc == 0), stop=(kc == NCT - 1))
            for hb in range(NP):
                nc.vector.tensor_copy(out=kT_sb[hb][:, 0:QB],
                                      in_=kps[hb][:])
            for ti in range(NQB):
                mms, evict = v_group(ti)
                for mm in mms:
                    mm()
                evict()
            for hb in range(NP):
                mms, evict = q_group(hb, 0)
                for mm in mms:
                    mm()
                evict()

            # queue remaining QKV work as filler
            for tch in range(1, NQB):
                for hb in range(NP):
                    filler.push(("K", tch), k_group(hb, tch))
                for hb in range(NP):
                    filler.push(("Q", tch), q_group(hb, tch))
                for ti in range(tch * 4, tch * 4 + 4):
                    filler.push(("V", tch), v_group(ti))

            # ---- attention chunk pipeline ----
            total_chunks = NP * sum(4 * (qb + 1) for qb in range(NQB))  # 160
            chunks_done = 0
            pending_pv = deque()

            class Blk:
                def __init__(self, hb, qb):
                    self.hb, self.qb = hb, qb
                    self.nkt = 4 * (qb + 1)
                    self.ps = None  # [even psum, odd psum]

            def emit_st_chunk(blk, kt):
                m = kt - 4 * blk.qb
                q0 = max(0, m * P)
                ps = ps_s.tile([P, 2 * QB], f32, tag="s", name="pss")
                for half in range(2):
                    base = half * D
                    nc.tensor.matmul(
                        ps[:, half * QB + q0:(half + 1) * QB],
                        lhsT=kT_sb[blk.hb][base:base + D, kt * P:(kt + 1) * P],
                        rhs=qT_sb[blk.hb][base:base + D,
                                          blk.qb * QB + q0:(blk.qb + 1) * QB],
                        start=True, stop=True)
                ch = cpool.tile([P, 2 * QB], bf, tag="ch", name="ch")
                if q0 == 0:
                    nc.scalar.activation(
                        out=ch[:], in_=ps[:],
                        func=mybir.ActivationFunctionType.Exp,
                        scale=float(1.0 / np.sqrt(D)))
                else:
                    # one ACT instruction for both halves via a 3D AP
                    ch3 = ch[:].rearrange("p (h q) -> p h q", q=QB)
                    ps3 = ps[:].rearrange("p (h q) -> p h q", q=QB)
                    nc.scalar.activation(
                        out=ch3[:, :, q0:QB], in_=ps3[:, :, q0:QB],
                        func=mybir.ActivationFunctionType.Exp,
                        scale=float(1.0 / np.sqrt(D)))
                if m >= 0:
                    for half in range(2):
                        dch = ch[:, half * QB + m * P:half * QB + (m + 1) * P]
                        nc.vector.tensor_tensor(
                            out=dch, in0=dch, in1=umask[:],
                            op=mybir.AluOpType.mult)
                return ch, q0

            def emit_pv_chunk(blk, kt, ch, q0):
                if blk.ps is None:
                    blk.ps = [ps_y.tile([D + 1, QB], f32, tag="y", name="psy0"),
                              ps_y.tile([D + 1, QB], f32, tag="y", name="psy1")]
                for he in range(2):
                    h = 2 * blk.hb + he
                    nc.tensor.matmul(
                        blk.ps[he][:, q0:QB],
                        lhsT=v65_sb[kt][:, h * (D + 1):(h + 1) * (D + 1)],
                        rhs=ch[:, he * QB + q0:(he + 1) * QB],
                        start=(kt == 0), stop=(kt == blk.nkt - 1))

            def emit_normalize(blk):
                # bf16 throughout: halves DVE stream cost (2x perf modes);
                # denominators are O(1e2-1e3) and y O(1), well within bf16.
                ytmp, rrow, rb = [], [], []
                for he in range(2):
                    ytmp.append(small.tile([D + 1, QB], bf, tag="ytmp",
                                           name="ytmp"))
                    nc.vector.tensor_copy(out=ytmp[he][:],
                                          in_=blk.ps[he][:, 0:QB])
                srow = []
                for he in range(2):
                    # reciprocal_approx_fast mis-lowers on HW when its input
                    # AP sits at base partition 64 — stage the denominator
                    # row through a partition-0 tile first. (fp32: the recip
                    # seed depends on fp32 bit layout.)
                    srow.append(small.tile([1, QB], f32, tag="srow",
                                           name="srow"))
                    nc.vector.tensor_copy(out=srow[he][:],
                                          in_=ytmp[he][D:D + 1, :])
                for he in range(2):
                    rrow.append(small.tile([1, QB], f32, tag="rrow",
                                           name="rrow"))
                    nc.vector.reciprocal_approx_fast(
                        rrow[he][:], srow[he][:])
                    rrowb = small.tile([1, QB], bf, tag="rrowb", name="rrowb")
                    nc.vector.tensor_copy(out=rrowb[:], in_=rrow[he][:])
                    rb.append(small.tile([D, QB], bf, tag="rb", name="rb"))
                    nc.gpsimd.partition_broadcast(rb[he][:], rrowb[:])
                for he in range(2):
                    nc.vector.tensor_tensor(
                        out=yT_sb[blk.hb][he * D:(he + 1) * D,
                                          blk.qb * QB:(blk.qb + 1) * QB],
                        in0=ytmp[he][0:D, :],
                        in1=rb[he][:],
                        op=mybir.AluOpType.mult)

            def pop_pv():
                blk, kt, ch, q0, last = pending_pv.popleft()
                emit_pv_chunk(blk, kt, ch, q0)
                if last:
                    emit_normalize(blk)
                    if blk.hb == NP - 1:
                        qb = blk.qb
                        for ti in range(qb * 4, qb * 4 + 4):
                            for co in range(C // QB):
                                filler.push(
                                    ("proj", qb),
                                    proj_group(ti, co,
                                               late=(2 if qb == 3 else
                                                     1 if qb == 2 else 0),
                                               split=False))

            for qb in range(NQB):
                if qb >= 1:
                    filler.drain_until(("Q", qb))
                    filler.drain_until(("V", qb))
                chunks_in_qb = NP * 4 * (qb + 1)
                chunks_in_qb_left = chunks_in_qb
                for hb in range(NP):
                    blk = Blk(hb, qb)
                    for kt in range(blk.nkt):
                        ch, q0 = emit_st_chunk(blk, kt)
                        pending_pv.append((blk, kt, ch, q0, kt == blk.nkt - 1))
                        chunks_done += 1
                        chunks_in_qb_left -= 1
                        if FINE_INTERLEAVE:
                            rem_chunks = total_chunks - chunks_done
                            if rem_chunks > 0:
                                avail = max(0, filler.total - TAIL_RESERVE_MMS)
                                want = -(-avail // rem_chunks)  # ceil
                                if qb < NQB - 1 and chunks_in_qb_left > 0:
                                    due = filler.mms_through(
                                        {("K", qb + 1), ("Q", qb + 1),
                                         ("V", qb + 1)})
                                    want = max(want,
                                               -(-due // chunks_in_qb_left))
                                while len(pending_pv) > LAG:
                                    pop_pv()
                                filler.pull(min(want, WANT_CAP))
                            while len(pending_pv) > LAG:
                                pop_pv()
                    if not FINE_INTERLEAVE:
                        while pending_pv:
                            pop_pv()
                        nblk = chunks_done // 10 + 1
                        avail = max(0, filler.total - TAIL_RESERVE_MMS)
                        est = max(8, avail * blk.nkt // max(1, total_chunks - chunks_done))
                        filler.pull_whole_groups(est)
            while pending_pv:
                pop_pv()
                filler.pull(DRAIN_PULL)
            filler.drain_all()
            if DBG:
                def dump(dst, tiles):
                    for i, tsb in enumerate(tiles):
                        sh = [tsb.shape[0], tsb.shape[1]]
                        tmp = dpool.tile(sh, f32, tag="d", name="dtmp")
                        nc.vector.tensor_copy(out=tmp[:], in_=tsb[:])
                        nc.sync.dma_start(out=dst.ap()[i], in_=tmp[:])
                dump(dbg_qT, qT_sb)
                dump(dbg_kT, kT_sb)
                dump(dbg_v, v65_sb)
                dump(dbg_yT, yT_sb)

    nc.compile()
    return nc


def _get_nc():
    global _BUILT
    if _BUILT is None:
        _BUILT = _build()
    return _BUILT


def _shard_inputs(x, w_attn, b_attn, w_proj):
    in_maps = []
    for c in range(NCORES):
        b, hh = divmod(c, 2)
        hoff = hh * CL
        # xT[tch][p, kc*512+t]
        xT = np.ascontiguousarray(
            x[b].T.reshape(NCT, P, NQB, QB).transpose(2, 1, 0, 3)
            .reshape(NQB, P, NCT * QB)
        ).astype(BF16)

        def wpack_hb(w):  # [C, CL] -> [P, hb*1024 + kc*128 + d]
            # w[kc*128+p, hb*128+d] -> out[p, hb*(NCT*128) + kc*128 + d]
            return np.ascontiguousarray(
                w.reshape(NCT, P, NP, P).transpose(1, 2, 0, 3)
                .reshape(P, NCT * CL)
            ).astype(BF16)

        def wpack(w):  # [C, CL] -> [P, kc*CL + oc]
            return np.ascontiguousarray(
                w.reshape(NCT, P, CL).transpose(1, 0, 2).reshape(P, NCT * CL)
            ).astype(BF16)

        wqkQ = wpack(w_attn[:, hoff:hoff + CL])
        wqkK = wpack(w_attn[:, C + hoff:C + hoff + CL])
        wv = wpack(w_attn[:, 2 * C + hoff:2 * C + hoff + CL])
        bqk = np.ascontiguousarray(
            np.concatenate(
                [b_attn[hoff:hoff + CL], b_attn[C + hoff:C + hoff + CL]]
            ).astype(np.float32).reshape((2 * CL) // P, P).T
        )
        wp = np.ascontiguousarray(
            w_proj[hoff:hoff + CL].reshape(CL // P, P, C)
            .transpose(1, 0, 2).reshape(P, (CL // P) * C)
        ).astype(BF16)
        in_maps.append(
            {"xT": xT, "wqkQ": wqkQ, "wqkK": wqkK, "wv": wv,
             "bqk": bqk, "wp": wp}
        )
    return in_maps


def _run(in_maps, trace=False):
    from concourse.bass_utils import run_bass_kernel_spmd

    nc = _get_nc()
    return run_bass_kernel_spmd(
        nc, in_maps, core_ids=list(range(NCORES)), trace=trace
    )


def kernel(x, w_attn, b_attn, w_proj, b_proj):
    x = np.asarray(x, dtype=np.float32)
    w_attn = np.asarray(w_attn, dtype=np.float32)
    b_attn = np.asarray(b_attn, dtype=np.float32)
    w_proj = np.asarray(w_proj, dtype=np.float32)
    b_proj = np.asarray(b_proj, dtype=np.float32)

    in_maps = _shard_inputs(x, w_attn, b_attn, w_proj)
    res = _run(in_maps)
    parts = [
        res.results[c]["out"].reshape(T, C).astype(np.float32)
        for c in range(NCORES)
    ]
    # v-bias commutes through the attention average (sum_k p_k = 1), so it
    # lands as an extra output bias: b_eff = b_proj + b_v @ w_proj.
    b_eff = b_proj + b_attn[2 * C:3 * C] @ w_proj
    out = np.stack(
        [parts[2 * b] + parts[2 * b + 1] + b_eff for b in range(B)]
    ).astype(np.float32)
    return out



# revision 50
# speedup vs baseline: 1.0104x; 1.0002x over previous
"""Causal self-attention (B=4, T=2048, C=1024, H=16, D=64) on 8 TRN2 NeuronCores.

Sharding: core c handles batch b = c//2 and head-half hh = c%2 (8 of 16 heads).
Each core computes its partial c_proj output [T, C] in bf16; the host sums the
two partials per batch and adds b_proj + b_v @ w_proj (the v-bias commutes
through the attention average since softmax weights sum to 1).

v3: single dense PE instruction stream instead of serial phases.
  - qb-major attention with a chunk-granular software pipeline:
    S^T chunk (2 matmuls, head pair) -> exp (ScalarE) -> PV lagging LAG chunks.
  - Head-pair row-packing: even head uses PE rows 0-63, odd head rows 64-127
    (auto tile_position) so the two K=64 S^T matmuls run concurrently on HW.
  - QKV projection / c_proj groups interleaved between attention chunks as
    filler so PE stays busy while ScalarE exponentiates.
  - Softmax denominator rides the PV matmul as a ones-column in the V tile;
    normalize is a lean bf16 DVE chain (copy/recip/bcast/mult), GpSimd does
    the partition broadcast.
  - Diagonal chunks exp both head-halves in ONE ScalarE instruction (3D AP).
  - Prologue K phase runs kc-outer across all four head-pairs (borrowing
    the idle ps_s banks as accumulators) so each arriving {wqkK, xT0} DMA
    block feeds 4 matmuls — compute matches the stream rate. PE warm-up
    matmuls run during the initial DMA latency window (HAM ramp).
  - bf16 output (halved eviction + DMA-out cost); host accumulates in f32.
"""

import sys
from collections import deque

for _p in ("/opt/trn_rl_repo", "/root/.axon_site"):
    if _p not in sys.path:
        sys.path.append(_p)

import numpy as np
import ml_dtypes

BF16 = ml_dtypes.bfloat16

B, T, C, H = 4, 2048, 1024, 16
D = C // H          # 64
NCORES = 8
HL = H // 2         # 8 local heads
NP = HL // 2        # 4 head pairs
CL = HL * D         # 512 local qkv channels
P = 128
TT = T // P         # 16 token tiles
QB = 512            # query block
NQB = T // QB       # 4
NCT = C // P        # 8 contraction tiles

LAG = 4
FINE_INTERLEAVE = True   # pull filler MMs between chunks (vs whole groups at block ends)
USE_LATE = True          # scalar-copy evicts + ps_s borrowing for tail proj groups
TAIL_RESERVE_MMS = 48
NCHUNK_BUFS = 8
WANT_CAP = 8             # max filler MMs pulled between consecutive chunks
DRAIN_PULL = 2           # filler MMs pulled per pop_pv in the final drain

_BUILT = None


def _build():
    import concourse.mybir as mybir
    import concourse.tile as tile
    from concourse import bacc
    from concourse.masks import make_upper_triangular

    bf = mybir.dt.bfloat16
    f32 = mybir.dt.float32

    nc = bacc.Bacc("TRN2", target_bir_lowering=False, debug=False,
                   num_devices=NCORES)

    # xT[tch][p, kc*512 + t]: token chunk tch, contraction block kc
    xT_d = nc.dram_tensor("xT", [NQB, P, NCT * QB], bf, kind="ExternalInput")
    # wqk*[p, kc*512 + oc]: contraction block kc, local out channel oc
    wqkQ_d = nc.dram_tensor("wqkQ", [P, NCT * CL], bf, kind="ExternalInput")
    wqkK_d = nc.dram_tensor("wqkK", [P, NCT * CL], bf, kind="ExternalInput")
    # wv[p, kc*512 + oc]: contraction block kc, local out channel oc
    wv_d = nc.dram_tensor("wv", [P, NCT * CL], bf, kind="ExternalInput")
    bqk_d = nc.dram_tensor("bqk", [P, (2 * CL) // P], f32, kind="ExternalInput")
    # wp[p, j*1024 + oc]
    wp_d = nc.dram_tensor("wp", [P, (CL // P) * C], bf, kind="ExternalInput")
    out_d = nc.dram_tensor("out", [TT, P, C], bf, kind="ExternalOutput")
    import os
    DBG = bool(os.environ.get("K2_DEBUG"))
    if DBG:
        dbg_qT = nc.dram_tensor("dbg_qT", [NP, P, T], f32, kind="ExternalOutput")
        dbg_kT = nc.dram_tensor("dbg_kT", [NP, P, T], f32, kind="ExternalOutput")
        dbg_v = nc.dram_tensor("dbg_v", [TT, P, HL * (D + 1)], f32,
                               kind="ExternalOutput")
        dbg_yT = nc.dram_tensor("dbg_yT", [CL // P, P, T], f32,
                                kind="ExternalOutput")

    with tile.TileContext(nc) as tc:
        with (
            tc.tile_pool(name="const", bufs=1) as const,
            tc.tile_pool(name="weights", bufs=1) as wpool,
            tc.tile_pool(name="acts", bufs=1) as apool,
            tc.tile_pool(name="chunks", bufs=NCHUNK_BUFS) as cpool,
            tc.tile_pool(name="outsb", bufs=6) as opool,
            tc.tile_pool(name="small", bufs=6) as small,
            tc.tile_pool(name="dbgpool", bufs=2) as dpool,
            tc.tile_pool(name="ps_s", bufs=2, space="PSUM") as ps_s,
            tc.tile_pool(name="ps_y", bufs=2, space="PSUM") as ps_y,
            tc.tile_pool(name="ps_mm", bufs=2, space="PSUM") as ps_mm,
        ):
            # ---- SBUF tensors ----
            wqkQ_sb = wpool.tile([P, NCT * CL], bf, tag="wqkQ", name="wqkQ")
            wqkK_sb = wpool.tile([P, NCT * CL], bf, tag="wqkK", name="wqkK")
            wv_sb = wpool.tile([P, NCT * CL], bf, tag="wv", name="wv")
            wp_sb = wpool.tile([P, (CL // P) * C], bf, tag="wp", name="wp")
            xT_sb = [apool.tile([P, NCT * QB], bf, tag=f"xTt{t}", name=f"xTt{t}")
                     for t in range(NQB)]
            qT_sb = [apool.tile([P, T], bf, tag=f"qT{b}", name=f"qT{b}") for b in range(NP)]
            kT_sb = [apool.tile([P, T], bf, tag=f"kT{b}", name=f"kT{b}") for b in range(NP)]
            v65_sb = [apool.tile([P, HL * (D + 1)], bf, tag=f"v65{ti}", name=f"v65{ti}")
                      for ti in range(TT)]
            yT_sb = [apool.tile([P, T], bf, tag=f"yT{j}", name=f"yT{j}") for j in range(CL // P)]

            # ---- DMA emission (ordered for earliest compute start) ----
            # The prologue K phase runs kc-outer over all four head-pairs,
            # so it consumes interleaved {wqkK, xT0} kc-blocks: each arriving
            # 512-col block feeds 4 matmuls (~852ns), matching the DMA
            # stream rate. wqkK is kc-major: cols [kc*512,(kc+1)*512) hold
            # all four head-pairs' weights for contraction tile kc.
            bqk_sb = const.tile([P, (2 * CL) // P], f32, tag="bqk")
            nc.sync.dma_start(out=xT_sb[0][:, 0:512], in_=xT_d.ap()[0][:, 0:512])
            nc.sync.dma_start(out=wqkK_sb[:, 0:128], in_=wqkK_d.ap()[:, 0:128])
            nc.sync.dma_start(out=wqkK_sb[:, 128:512],
                              in_=wqkK_d.ap()[:, 128:512])
            nc.sync.dma_start(out=wqkK_sb[:, 512:1536],
                              in_=wqkK_d.ap()[:, 512:1536])
            nc.sync.dma_start(out=xT_sb[0][:, 512:1536],
                              in_=xT_d.ap()[0][:, 512:1536])
            nc.sync.dma_start(out=wqkK_sb[:, 1536:2560],
                              in_=wqkK_d.ap()[:, 1536:2560])
            nc.sync.dma_start(out=xT_sb[0][:, 1536:2560],
                              in_=xT_d.ap()[0][:, 1536:2560])
            nc.sync.dma_start(out=wqkK_sb[:, 2560:3584],
                              in_=wqkK_d.ap()[:, 2560:3584])
            nc.sync.dma_start(out=xT_sb[0][:, 2560:3584],
                              in_=xT_d.ap()[0][:, 2560:3584])
            nc.sync.dma_start(out=wqkK_sb[:, 3584:4096],
                              in_=wqkK_d.ap()[:, 3584:4096])
            nc.sync.dma_start(out=xT_sb[0][:, 3584:4096],
                              in_=xT_d.ap()[0][:, 3584:4096])

            # ---- constants / act-table preload (off the critical DMA path) ----
            warm = const.tile([1, 16], f32, tag="warm")
            nc.vector.memset(warm[:], 0.0)
            wexp = const.tile([1, 16], f32, tag="wexp")
            nc.scalar.activation(out=wexp[:], in_=warm[:],
                                 func=mybir.ActivationFunctionType.Exp)
            # PE warm-up during the initial DMA latency window: keeps the
            # HAM activity monitor busy so real matmuls start at full clock.
            warm_bf = const.tile([1, 64], bf, tag="warmbf")
            nc.vector.memset(warm_bf[:], 0.0)
            ps_warm = ps_mm.tile([P, QB], f32, tag="mm", name="warmps")
            for _ in range(12):
                nc.tensor.matmul(ps_warm[0:64, 0:64],
                                 lhsT=warm_bf[:, 0:64],
                                 rhs=warm_bf[:, 0:64],
                                 start=True, stop=True)
            umask = const.tile([P, P], bf, tag="umask")
            make_upper_triangular(nc, umask[:], val=1.0, diag=True)
            nc.sync.dma_start(out=wv_sb[:, 0:1024], in_=wv_d.ap()[:, 0:1024])
            nc.sync.dma_start(out=wv_sb[:, 1024:2560],
                              in_=wv_d.ap()[:, 1024:2560])
            nc.sync.dma_start(out=wv_sb[:, 2560:4096],
                              in_=wv_d.ap()[:, 2560:4096])
            nc.sync.dma_start(out=bqk_sb[:], in_=bqk_d.ap())
            nc.sync.dma_start(out=wqkQ_sb[:], in_=wqkQ_d.ap())
            for tch in range(1, NQB):
                nc.sync.dma_start(out=xT_sb[tch][:], in_=xT_d.ap()[tch])
            nc.sync.dma_start(out=wp_sb[:], in_=wp_d.ap())

            # ---- group emitters (each: list of MM closures + evict) ----
            def k_group(hb, tch):
                ps = [None]
                def mk(kc):
                    def mm():
                        if ps[0] is None:
                            ps[0] = ps_mm.tile([P, QB], f32, tag="mm", name="mm")
                        nc.tensor.matmul(
                            ps[0][:],
                            lhsT=wqkK_sb[:, kc * CL + hb * P:
                                            kc * CL + (hb + 1) * P],
                            rhs=xT_sb[tch][:, kc * QB:(kc + 1) * QB],
                            start=(kc == 0), stop=(kc == NCT - 1))
                    return mm
                def evict():
                    # No k-bias: for a fixed query, (q+bq)·bk is constant
                    # across keys, so it cancels in softmax — only bq enters
                    # (via bq·k). Exact equivalence; plain copy eviction.
                    nc.vector.tensor_copy(
                        out=kT_sb[hb][:, tch * QB:(tch + 1) * QB],
                        in_=ps[0][:])
                return [mk(kc) for kc in range(NCT)], evict

            def q_group(hb, tch):
                ps = [None]
                def mk(kc):
                    def mm():
                        if ps[0] is None:
                            ps[0] = ps_mm.tile([P, QB], f32, tag="mm", name="mm")
                        nc.tensor.matmul(
                            ps[0][:],
                            lhsT=wqkQ_sb[:, kc * CL + hb * P:
                                            kc * CL + (hb + 1) * P],
                            rhs=xT_sb[tch][:, kc * QB:(kc + 1) * QB],
                            start=(kc == 0), stop=(kc == NCT - 1))
                    return mm
                def evict():
                    nc.vector.tensor_scalar_add(
                        qT_sb[hb][:, tch * QB:(tch + 1) * QB],
                        ps[0][:], bqk_sb[:, hb:hb + 1])
                return [mk(kc) for kc in range(NCT)], evict

            def v_group(ti):
                ps = [None]
                def mk(kc):
                    def mm():
                        if ps[0] is None:
                            ps[0] = ps_mm.tile([P, QB], f32, tag="mm", name="mm")
                        nc.tensor.matmul(
                            ps[0][:],
                            lhsT=xT_sb[ti // 4][:, kc * QB + (ti % 4) * P:
                                                  kc * QB + (ti % 4 + 1) * P],
                            rhs=wv_sb[:, kc * CL:(kc + 1) * CL],
                            start=(kc == 0), stop=(kc == NCT - 1))
                    return mm
                def evict():
                    # v65 layout per head: cols 0..63 = v, col 64 = ones (the
                    # softmax denominator rides the PV matmul). v-bias is
                    # folded into the host-side output bias (sum_k p_k = 1).
                    # Copy on ScalarE: keeps the forced-drain eviction chain
                    # off DVE (ps_mm reuse is gated by evictions there).
                    v3 = v65_sb[ti][:].rearrange("p (h e) -> p h e", e=D + 1)
                    nc.scalar.copy(
                        out=v3[:, :, 0:D],
                        in_=ps[0][:].rearrange("p (h e) -> p h e", e=D))
                    nc.vector.memset(v3[:, :, D:D + 1], 1.0)
                return [mk(kc) for kc in range(NCT)], evict

            def proj_group(ti, co, late=False, split=False):
                ps = [None]
                if split and USE_LATE:
                    # Final group: two 256-wide column-halves accumulated in
                    # DIFFERENT PSUM banks of one borrowed ps_s tile, so half
                    # A's evict+DMA overlaps half B's matmuls and only a
                    # 256-wide chain trails the last matmul (kernel tail).
                    hq = QB // 2
                    def evict_half(half, eng):
                        osb = opool.tile([P, hq], bf, tag="o", name="osb")
                        if eng == "act":
                            nc.scalar.copy(out=osb[:], in_=ps[0][half][:])
                        else:
                            nc.vector.tensor_copy(out=osb[:],
                                                  in_=ps[0][half][:])
                        nc.sync.dma_start(
                            out=out_d.ap()[ti][:, co * QB + half * hq:
                                               co * QB + (half + 1) * hq],
                            in_=osb[:])
                    def mk2(idx):
                        def mm():
                            if ps[0] is None:
                                # separate tiles from separate pools: Tile's
                                # dependency tracker is tile-granular, and
                                # the halves must not gate on one rotation.
                                bigA = ps_s.tile([P, 2 * QB], f32, tag="s",
                                                 name="pss")
                                psB = ps_mm.tile([P, QB], f32, tag="mm",
                                                 name="mm")
                                ps[0] = [bigA[:, 0:hq], psB[:, 0:hq]]
                            half, j = divmod(idx, CL // P)
                            nc.tensor.matmul(
                                ps[0][half][:],
                                lhsT=yT_sb[j][:, ti * P:(ti + 1) * P],
                                rhs=wp_sb[:, j * C + co * QB + half * hq:
                                           j * C + co * QB + half * hq + hq],
                                start=(j == 0), stop=(j == CL // P - 1))
                            if idx == CL // P - 1:
                                evict_half(0, "dve")
                        return mm
                    def evict():
                        evict_half(1, "act")
                    return [mk2(i) for i in range(2 * (CL // P))], evict
                def mk(j):
                    def mm():
                        if ps[0] is None:
                            if USE_LATE and late == 2 and (ti * 2 + co) % 2 == 0:
                                big = ps_s.tile([P, 2 * QB], f32, tag="s",
                                                name="pss")
                                ps[0] = big[:, 0:QB]
                            else:
                                ps[0] = ps_mm.tile([P, QB], f32, tag="mm",
                                                   name="mm")
                        nc.tensor.matmul(
                            ps[0][:],
                            lhsT=yT_sb[j][:, ti * P:(ti + 1) * P],
                            rhs=wp_sb[:, j * C + co * QB:j * C + (co + 1) * QB],
                            start=(j == 0), stop=(j == CL // P - 1))
                    return mm
                def evict():
                    osb = opool.tile([P, QB], bf, tag="o", name="osb")
                    if late and USE_LATE:
                        nc.scalar.copy(out=osb[:], in_=ps[0][:])
                    else:
                        nc.vector.tensor_copy(out=osb[:], in_=ps[0][:])
                    nc.sync.dma_start(
                        out=out_d.ap()[ti][:, co * QB:(co + 1) * QB],
                        in_=osb[:])
                return [mk(j) for j in range(CL // P)], evict

            # ---- filler machinery ----
            class Filler:
                def __init__(self):
                    self.q = deque()   # (label, mms list, evict)
                    self.total = 0
                def push(self, label, group):
                    mms, evict = group
                    self.q.append((label, list(mms), evict))
                    self.total += len(mms)
                def _step(self):
                    label, mms, evict = self.q[0]
                    mms.pop(0)()
                    self.total -= 1
                    if not mms:
                        evict()
                        self.q.popleft()
                def pull(self, n):
                    for _ in range(n):
                        if not self.q:
                            return
                        self._step()
                def drain_until(self, label):
                    while self.q and any(l == label for l, _, _ in self.q):
                        self._step()
                def mms_through(self, labels):
                    n, upto = 0, 0
                    for i, (l, mms, _) in enumerate(self.q):
                        n += len(mms)
                        if l in labels:
                            upto = n
                    return upto
                def drain_all(self):
                    while self.q:
                        self._step()
                def pull_whole_groups(self, n_mms):
                    done = 0
                    while self.q and done < n_mms:
                        done += len(self.q[0][1])
                        while self.q and self.q[0][1]:
                            self._step()

            filler = Filler()

            # ---- prologue: QKV for token chunk 0 ----
            # K phase kc-outer: all four head-pairs accumulate per kc block,
            # borrowing ps_s (idle until attention) as the four accumulators
            # (two [128,1024] tiles, one bank per head-pair half). Each
            # arriving DMA block is consumed 4x, matching the stream rate.
            kpsA = ps_s.tile([P, 2 * QB], f32, tag="s", name="pss")
            kpsB = ps_s.tile([P, 2 * QB], f32, tag="s", name="pss")
            kps = [kpsA[:, 0:QB], kpsA[:, QB:2 * QB],
                   kpsB[:, 0:QB], kpsB[:, QB:2 * QB]]
            for kc in range(NCT):
                for hb in range(NP):
                    nc.tensor.matmul(
                        kps[hb][:],
                        lhsT=wqkK_sb[:, kc * CL + hb * P:kc * CL + (hb + 1) * P],
                        rhs=xT_sb[0][:, kc * QB:(kc + 1) * QB],
                        start=(kc == 0), stop=(kc == NCT - 1))
            for hb in range(NP):
                nc.vector.tensor_copy(out=kT_sb[hb][:, 0:QB],
                                      in_=kps[hb][:])
            for ti in range(NQB):
                mms, evict = v_group(ti)
                for mm in mms:
                    mm()
                evict()
            for hb in range(NP):
                mms, evict = q_group(hb, 0)
                for mm in mms:
                    mm()
                evict()

            # queue remaining QKV work as filler
            for tch in range(1, NQB):
                for hb in range(NP):
                    filler.push(("K", tch), k_group(hb, tch))
                for hb in range(NP):
                    filler.push(("Q", tch), q_group(hb, tch))
                for ti in range(tch * 4, tch * 4 + 4):
                    filler.push(("V", tch), v_group(ti))

            # ---- attention chunk pipeline ----
            total_chunks = NP * sum(4 * (qb + 1) for qb in range(NQB))  # 160
            chunks_done = 0
            pending_pv = deque()

            class Blk:
                def __init__(self, hb, qb):
                    self.hb, self.qb = hb, qb
                    self.nkt = 4 * (qb + 1)
                    self.ps = None  # [even psum, odd psum]

            def emit_st_chunk(blk, kt):
                m = kt - 4 * blk.qb
                q0 = max(0, m * P)
                ps = ps_s.tile([P, 2 * QB], f32, tag="s", name="pss")
                for half in range(2):
                    base = half * D
                    nc.tensor.matmul(
                        ps[:, half * QB + q0:(half + 1) * QB],
                        lhsT=kT_sb[blk.hb][base:base + D, kt * P:(kt + 1) * P],
                        rhs=qT_sb[blk.hb][base:base + D,
                                          blk.qb * QB + q0:(blk.qb + 1) * QB],
                        start=True, stop=True)
                ch = cpool.tile([P, 2 * QB], bf, tag="ch", name="ch")
                if q0 == 0:
                    nc.scalar.activation(
                        out=ch[:], in_=ps[:],
                        func=mybir.ActivationFunctionType.Exp,
                        scale=float(1.0 / np.sqrt(D)))
                else:
                    # one ACT instruction for both halves via a 3D AP
                    ch3 = ch[:].rearrange("p (h q) -> p h q", q=QB)
                    ps3 = ps[:].rearrange("p (h q) -> p h q", q=QB)
                    nc.scalar.activation(
                        out=ch3[:, :, q0:QB], in_=ps3[:, :, q0:QB],
                        func=mybir.ActivationFunctionType.Exp,
                        scale=float(1.0 / np.sqrt(D)))
                if m >= 0:
                    for half in range(2):
                        dch = ch[:, half * QB + m * P:half * QB + (m + 1) * P]
                        nc.vector.tensor_tensor(
                            out=dch, in0=dch, in1=umask[:],
                            op=mybir.AluOpType.mult)
                return ch, q0

            def emit_pv_chunk(blk, kt, ch, q0):
                if blk.ps is None:
                    blk.ps = [ps_y.tile([D + 1, QB], f32, tag="y", name="psy0"),
                              ps_y.tile([D + 1, QB], f32, tag="y", name="psy1")]
                for he in range(2):
                    h = 2 * blk.hb + he
                    nc.tensor.matmul(
                        blk.ps[he][:, q0:QB],
                        lhsT=v65_sb[kt][:, h * (D + 1):(h + 1) * (D + 1)],
                        rhs=ch[:, he * QB + q0:(he + 1) * QB],
                        start=(kt == 0), stop=(kt == blk.nkt - 1))

            def emit_normalize(blk):
                # bf16 throughout: halves DVE stream cost (2x perf modes);
                # denominators are O(1e2-1e3) and y O(1), well within bf16.
                # For the FINAL block the chain gates the tail proj groups:
                # route the he=1 copies through ScalarE (its exps are done)
                # so the two half-chains overlap.
                last = (blk.qb == NQB - 1 and blk.hb == NP - 1)
                ytmp, rrow, rb = [], [], []
                for he in range(2):
                    ytmp.append(small.tile([D + 1, QB], bf, tag="ytmp",
                                           name="ytmp"))
                    if last and he == 1:
                        nc.scalar.copy(out=ytmp[he][:],
                                       in_=blk.ps[he][:, 0:QB])
                    else:
                        nc.vector.tensor_copy(out=ytmp[he][:],
                                              in_=blk.ps[he][:, 0:QB])
                srow = []
                for he in range(2):
                    # reciprocal_approx_fast mis-lowers on HW when its input
                    # AP sits at base partition 64 — stage the denominator
                    # row through a partition-0 tile first. (fp32: the recip
                    # seed depends on fp32 bit layout.)
                    srow.append(small.tile([1, QB], f32, tag="srow",
                                           name="srow"))
                    if last and he == 1:
                        nc.scalar.copy(out=srow[he][:],
                                       in_=ytmp[he][D:D + 1, :])
                    else:
                        nc.vector.tensor_copy(out=srow[he][:],
                                              in_=ytmp[he][D:D + 1, :])
                for he in range(2):
                    rrow.append(small.tile([1, QB], f32, tag="rrow",
                                           name="rrow"))
                    nc.vector.reciprocal_approx_fast(
                        rrow[he][:], srow[he][:])
                    rrowb = small.tile([1, QB], bf, tag="rrowb", name="rrowb")
                    nc.vector.tensor_copy(out=rrowb[:], in_=rrow[he][:])
                    rb.append(small.tile([D, QB], bf, tag="rb", name="rb"))
                    nc.gpsimd.partition_broadcast(rb[he][:], rrowb[:])
                for he in range(2):
                    nc.vector.tensor_tensor(
                        out=yT_sb[blk.hb][he * D:(he + 1) * D,
                                          blk.qb * QB:(blk.qb + 1) * QB],
                        in0=ytmp[he][0:D, :],
                        in1=rb[he][:],
                        op=mybir.AluOpType.mult)

            def pop_pv():
                blk, kt, ch, q0, last = pending_pv.popleft()
                emit_pv_chunk(blk, kt, ch, q0)
                if last:
                    emit_normalize(blk)
                    if blk.hb == NP - 1:
                        qb = blk.qb
                        for ti in range(qb * 4, qb * 4 + 4):
                            for co in range(C // QB):
                                filler.push(
                                    ("proj", qb),
                                    proj_group(ti, co,
                                               late=(2 if qb == 3 else
                                                     1 if qb == 2 else 0),
                                               split=False))

            for qb in range(NQB):
                if qb >= 1:
                    filler.drain_until(("Q", qb))
                    filler.drain_until(("V", qb))
                chunks_in_qb = NP * 4 * (qb + 1)
                chunks_in_qb_left = chunks_in_qb
                for hb in range(NP):
                    blk = Blk(hb, qb)
                    for kt in range(blk.nkt):
                        ch, q0 = emit_st_chunk(blk, kt)
                        pending_pv.append((blk, kt, ch, q0, kt == blk.nkt - 1))
                        chunks_done += 1
                        chunks_in_qb_left -= 1
                        if FINE_INTERLEAVE:
                            rem_chunks = total_chunks - chunks_done
                            if rem_chunks > 0:
                                avail = max(0, filler.total - TAIL_RESERVE_MMS)
                                want = -(-avail // rem_chunks)  # ceil
                                if qb < NQB - 1 and chunks_in_qb_left > 0:
                                    due = filler.mms_through(
                                        {("K", qb + 1), ("Q", qb + 1),
                                         ("V", qb + 1)})
                                    want = max(want,
                                               -(-due // chunks_in_qb_left))
                                while len(pending_pv) > LAG:
                                    pop_pv()
                                filler.pull(min(want, WANT_CAP))
                            while len(pending_pv) > LAG:
                                pop_pv()
                    if not FINE_INTERLEAVE:
                        while pending_pv:
                            pop_pv()
                        nblk = chunks_done // 10 + 1
                        avail = max(0, filler.total - TAIL_RESERVE_MMS)
                        est = max(8, avail * blk.nkt // max(1, total_chunks - chunks_done))
                        filler.pull_whole_groups(est)
            while pending_pv:
                pop_pv()
                filler.pull(DRAIN_PULL)
            filler.drain_all()
            if DBG:
                def dump(dst, tiles):
                    for i, tsb in enumerate(tiles):
                        sh = [tsb.shape[0], tsb.shape[1]]
                        tmp = dpool.tile(sh, f32, tag="d", name="dtmp")
                        nc.vector.tensor_copy(out=tmp[:], in_=tsb[:])
                        nc.sync.dma_start(out=dst.ap()[i], in_=tmp[:])
                dump(dbg_qT, qT_sb)
                dump(dbg_kT, kT_sb)
                dump(dbg_v, v65_sb)
                dump(dbg_yT, yT_sb)

    nc.compile()
    return nc


def _get_nc():
    global _BUILT
    if _BUILT is None:
        _BUILT = _build()
    return _BUILT


def _shard_inputs(x, w_attn, b_attn, w_proj):
    in_maps = []
    for c in range(NCORES):
        b, hh = divmod(c, 2)
        hoff = hh * CL
        # xT[tch][p, kc*512+t]
        xT = np.ascontiguousarray(
            x[b].T.reshape(NCT, P, NQB, QB).transpose(2, 1, 0, 3)
            .reshape(NQB, P, NCT * QB)
        ).astype(BF16)

        def wpack(w):  # [C, CL] -> [P, kc*CL + oc]
            return np.ascontiguousarray(
                w.reshape(NCT, P, CL).transpose(1, 0, 2).reshape(P, NCT * CL)
            ).astype(BF16)

        wqkQ = wpack(w_attn[:, hoff:hoff + CL])
        wqkK = wpack(w_attn[:, C + hoff:C + hoff + CL])
        wv = wpack(w_attn[:, 2 * C + hoff:2 * C + hoff + CL])
        bqk = np.ascontiguousarray(
            np.concatenate(
                [b_attn[hoff:hoff + CL], b_attn[C + hoff:C + hoff + CL]]
            ).astype(np.float32).reshape((2 * CL) // P, P).T
        )
        wp = np.ascontiguousarray(
            w_proj[hoff:hoff + CL].reshape(CL // P, P, C)
            .transpose(1, 0, 2).reshape(P, (CL // P) * C)
        ).astype(BF16)
        in_maps.append(
            {"xT": xT, "wqkQ": wqkQ, "wqkK": wqkK, "wv": wv,
             "bqk": bqk, "wp": wp}
        )
    return in_maps


def _run(in_maps, trace=False):
    from concourse.bass_utils import run_bass_kernel_spmd

    nc = _get_nc()
    return run_bass_kernel_spmd(
        nc, in_maps, core_ids=list(range(NCORES)), trace=trace
    )


def kernel(x, w_attn, b_attn, w_proj, b_proj):
    x = np.asarray(x, dtype=np.float32)
    w_attn = np.asarray(w_attn, dtype=np.float32)
    b_attn = np.asarray(b_attn, dtype=np.float32)
    w_proj = np.asarray(w_proj, dtype=np.float32)
    b_proj = np.asarray(b_proj, dtype=np.float32)

    in_maps = _shard_inputs(x, w_attn, b_attn, w_proj)
    res = _run(in_maps)
    parts = [
        res.results[c]["out"].reshape(T, C).astype(np.float32)
        for c in range(NCORES)
    ]
    # v-bias commutes through the attention average (sum_k p_k = 1), so it
    # lands as an extra output bias: b_eff = b_proj + b_v @ w_proj.
    b_eff = b_proj + b_attn[2 * C:3 * C] @ w_proj
    out = np.stack(
        [parts[2 * b] + parts[2 * b + 1] + b_eff for b in range(B)]
    ).astype(np.float32)
    return out

